# revision 1
# baseline (speedup 1.0000x reference)
"""Trainium2 Bass kernel for nn_NodeClassifier (gnn_message_passing).

Strategy (8 NeuronCores, SPMD):
  - Nodes block-partitioned by id across 8 cores (6250 each, padded to 6272).
    Within each core's block, nodes are sorted by in-degree so that the
    padded neighbor grid (K-grid) is tight.
  - Edges partitioned by dst core. Per dst node, its neighbor src tokens are
    laid out in a [128 nodes x K_t] index grid per 128-node tile; the src
    embeddings are fetched with indirect DMA gathers from a replicated
    node-embedding table in DRAM, then tree-reduced on the vector engine.
  - All per-node dense compute (GCN linear, BN, FF) runs feature-major
    ([128 features x nodes]) on the core's own shard. BN statistics are
    AllReduced (tiny). Between layers, the post-BN embeddings are AllGathered
    so every core has the full table for the next layer's gathers.
  - Weights replicated.

The program is identical on all cores (shared schedules = max over cores);
per-core behavior comes only from per-core input arrays.
"""

import os
import sys
import numpy as np

for _p in ("/opt/trn_rl_repo",):
    if _p not in sys.path and os.path.isdir(_p):
        sys.path.insert(0, _p)

from contextlib import ExitStack

import concourse.bass as bass
import concourse.bacc as bacc
import concourse.mybir as mybir
import concourse.tile as tile
from concourse.bass import IndirectOffsetOnAxis
from concourse.bass_utils import run_bass_kernel_spmd
from concourse.masks import make_identity

F32 = mybir.dt.float32
F16 = mybir.dt.float16
F32R = mybir.dt.float32r
I32 = mybir.dt.int32
AF = mybir.ActivationFunctionType
ALU = mybir.AluOpType

CORES = 8
D = 128
H = 512
DEPTH = 2
EPS = 1e-5
CHUNK = 512  # node-chunk width for the dense phase (one PSUM bank fp32)


# ----------------------------------------------------------------------------
# Host-side preparation
# ----------------------------------------------------------------------------

def _prepare(nodes, edge_src, edge_dst):
    """Compute the permutation, sharding and gather schedules from edge data."""
    N = nodes.shape[0]
    assert N % CORES == 0
    sh_real = N // CORES
    nt = -(-sh_real // 128)
    sh = nt * 128
    if sh == sh_real:  # force at least one dummy slot (PAD token row must be 0)
        nt += 1
        sh += 128
    tok_n = CORES * sh

    deg = np.bincount(edge_dst, minlength=N).astype(np.int64)

    # permutation: per core block, sort nodes by degree ascending
    tok_of_node = np.empty(N, np.int64)
    node_of_tok = np.full(tok_n, -1, np.int64)
    for c in range(CORES):
        ids = np.arange(c * sh_real, (c + 1) * sh_real)
        order = np.argsort(deg[ids], kind="stable")
        toks = c * sh + np.arange(sh_real)
        tok_of_node[ids[order]] = toks
        node_of_tok[toks] = ids[order]

    pad_tok = sh_real  # core 0's first dummy slot; its table row is zero

    # group edges by dst token
    dst_tok = tok_of_node[edge_dst]
    src_tok = tok_of_node[edge_src]
    order = np.argsort(dst_tok, kind="stable")
    dst_tok_s = dst_tok[order]
    src_tok_s = src_tok[order]
    cnt_tok = np.bincount(dst_tok_s, minlength=tok_n)
    start_tok = np.concatenate([[0], np.cumsum(cnt_tok)[:-1]])

    # shared K schedule: per tile index t, max over cores of max degree, even
    K_t = np.zeros(nt, np.int64)
    cnt_mat = cnt_tok.reshape(CORES, nt, 128)
    K_t = cnt_mat.max(axis=(0, 2))
    K_t = np.maximum(K_t, 2)
    K_t = K_t + (K_t % 2)
    koff = np.concatenate([[0], np.cumsum(K_t)])
    ksum = int(koff[-1])

    # per-core gather index grids [128, ksum] int32 (partition = node slot%128)
    gidx = np.full((CORES, 128, ksum), pad_tok, np.int32)
    t_of_slot = np.arange(sh) // 128
    e_slot = dst_tok_s % sh  # slot within core
    e_core = dst_tok_s // sh
    e_t = e_slot // 128
    e_p = e_slot % 128
    e_r = np.arange(len(dst_tok_s)) - start_tok[dst_tok_s]  # rank within node
    e_col = koff[e_t] + e_r
    gidx[e_core, e_p, e_col] = src_tok_s

    # per-core invdeg [128, nt] (0 for dummy slots)
    invdeg = np.zeros((CORES, 128, nt), np.float32)
    deg_tok = cnt_tok.reshape(CORES, sh)
    node_ok = (node_of_tok.reshape(CORES, sh) >= 0)
    iv = 1.0 / np.maximum(deg_tok, 1.0)
    iv = iv * node_ok
    for c in range(CORES):
        invdeg[c] = iv[c].reshape(nt, 128).T

    # replicated full node table [tok_n, D], zero at dummy slots
    table0 = np.zeros((tok_n, D), np.float32)
    real = node_of_tok >= 0
    table0[real] = nodes[node_of_tok[real]]

    # host-expanded layer-1 gather payload, fp16 [CORES][128, ksum*D]
    t16 = table0.astype(np.float16)
    pay1 = t16[gidx]  # [CORES, 128, ksum, D]
    pay1 = np.ascontiguousarray(pay1.reshape(CORES, 128, ksum * D))

    return dict(
        N=N, sh_real=sh_real, sh=sh, nt=nt, tok_n=tok_n,
        K_t=[int(k) for k in K_t], koff=[int(k) for k in koff], ksum=ksum,
        gidx=gidx, invdeg=invdeg, table0=table0, pay1=pay1,
        node_of_tok=node_of_tok,
    )


# ----------------------------------------------------------------------------
# Program builder
# ----------------------------------------------------------------------------

def _emit_tree_reduce(nc, G16, G32, K, acc):
    """acc = sum of K [128,D] fp16 chunks of G16. Pass 1 pairs fp16 halves
    into fp32 G32, then in-place fp32 halving on G32."""
    half = K // 2  # K is even
    if half == 1:
        nc.vector.tensor_tensor(out=acc[:], in0=G16[:, :D],
                                in1=G16[:, D:2 * D], op=ALU.add)
        return
    nc.vector.tensor_tensor(out=G32[:, :half * D], in0=G16[:, :half * D],
                            in1=G16[:, half * D:K * D], op=ALU.add)
    width = half
    while width > 2:
        h = width // 2
        if width % 2:
            nc.vector.tensor_tensor(
                out=G32[:, 0:D], in0=G32[:, 0:D],
                in1=G32[:, (width - 1) * D:width * D], op=ALU.add)
        if h == 1:  # width was 3: after the fold only chunks 0,1 remain
            break
        nc.vector.tensor_tensor(
            out=G32[:, :h * D], in0=G32[:, :h * D],
            in1=G32[:, h * D:2 * h * D], op=ALU.add)
        width = h
    nc.vector.tensor_tensor(out=acc[:], in0=G32[:, 0:D], in1=G32[:, D:2 * D],
                            op=ALU.add)


def build_program(cfg, debug=False):
    nt, sh, sh_real = cfg["nt"], cfg["sh"], cfg["sh_real"]
    tok_n, ksum = cfg["tok_n"], cfg["ksum"]
    K_t, koff = cfg["K_t"], cfg["koff"]
    N = cfg["N"]
    kmax = max(K_t)
    rg = [list(range(CORES))]

    chunks = []
    c0 = 0
    while c0 < sh:
        cw = min(CHUNK, sh - c0)
        chunks.append((c0, cw))
        c0 += cw
    nch = len(chunks)

    nc = bacc.Bacc("TRN2", target_bir_lowering=False, debug=False,
                   num_devices=CORES)

    # ---- I/O declarations
    pay1_d = nc.dram_tensor("pay1", [128, ksum * D], F16, kind="ExternalInput")
    x0_d = nc.dram_tensor("x0_fm", [D, sh], F32, kind="ExternalInput")
    gidx_d = nc.dram_tensor("gidx", [128, ksum], I32, kind="ExternalInput")
    invdeg_d = nc.dram_tensor("invdeg", [128, nt], F32, kind="ExternalInput")
    wg_d = [nc.dram_tensor(f"wg{l}", [D, D], F32, kind="ExternalInput")
            for l in range(DEPTH)]
    bgT_d = [nc.dram_tensor(f"bgT{l}", [1, D], F32, kind="ExternalInput")
             for l in range(DEPTH)]
    w1_d = [nc.dram_tensor(f"w1_{l}", [D, H], F32, kind="ExternalInput")
            for l in range(DEPTH)]
    fb1_d = [nc.dram_tensor(f"fb1_{l}", [D, H // D], F32, kind="ExternalInput")
             for l in range(DEPTH)]
    w2_d = [nc.dram_tensor(f"w2_{l}", [H, D], F32, kind="ExternalInput")
            for l in range(DEPTH)]
    bn_d = {}
    for l in range(DEPTH):
        for nm in ("g1", "b1", "g2", "b2"):
            bn_d[(nm, l)] = nc.dram_tensor(f"{nm}_{l}", [D, 1], F32,
                                           kind="ExternalInput")
    clsw_d = nc.dram_tensor("clsw", [D, 16], F32, kind="ExternalInput")
    clsb_d = nc.dram_tensor("clsb", [16, 1], F32, kind="ExternalInput")
    out_d = nc.dram_tensor("out_fm", [16, sh], F32, kind="ExternalOutput")
    dbg = {}
    if debug:
        for nm, shape, dt_ in [("dbg_agg0", [D, sh], F32),
                               ("dbg_u0", [D, sh], F32),
                               ("dbg_s2", [D, 2], F32),
                               ("dbg_sums", [D, 2], F32),
                               ("dbg_a1c1", [D, 2], F32),
                               ("dbg_v0", [D, sh], F32),
                               ("dbg_xnew0", [D, sh], F32),
                               ("dbg_vtab", [cfg["tok_n"], D], F16),
                               ("dbg_agg1", [D, sh], F32)]:
            dbg[nm] = nc.dram_tensor(nm, shape, dt_, kind="ExternalOutput")

    with tile.TileContext(nc) as tc, ExitStack() as ctx:
        dram = ctx.enter_context(tc.tile_pool(name="dram", bufs=1, space="DRAM"))
        wp = ctx.enter_context(tc.tile_pool(name="weights", bufs=1))
        big = ctx.enter_context(tc.tile_pool(name="big", bufs=1))
        gp = ctx.enter_context(tc.tile_pool(name="gather", bufs=2))
        sp = ctx.enter_context(tc.tile_pool(name="small", bufs=4))
        ck = ctx.enter_context(tc.tile_pool(name="chunk", bufs=2))
        psA = ctx.enter_context(tc.tile_pool(name="psA", bufs=1, space="PSUM"))
        psB = ctx.enter_context(tc.tile_pool(name="psB", bufs=2, space="PSUM"))

        # ---- internal DRAM (collective bounce buffers)
        vshard = dram.tile([sh, D], F16, name="vshard")
        vtab = dram.tile([tok_n, D], F16, addr_space="Shared", name="vtab")
        bn_in, bn_out = {}, {}
        for l in range(DEPTH):
            for j in (1, 2):
                bn_in[(l, j)] = dram.tile([D, 2], F32, name=f"bni{l}{j}")
                bn_out[(l, j)] = dram.tile([D, 2], F32, addr_space="Shared",
                                           name=f"bno{l}{j}")

        # ---- load constants / weights to SBUF
        def load(dt_, shape, src, name):
            t = wp.tile(shape, dt_, name=name)
            nc.sync.dma_start(out=t[:], in_=src)
            return t

        gidx_sb = load(I32, [128, ksum], gidx_d[:], "gidx_sb")
        invdeg_sb = load(F32, [128, nt], invdeg_d[:], "invdeg_sb")
        wg_sb = [load(F32, [D, D], wg_d[l][:], f"wg_sb{l}") for l in range(DEPTH)]
        bgT_sb = [load(F32, [1, D], bgT_d[l][:], f"bgT_sb{l}") for l in range(DEPTH)]
        w1_sb = [load(F32, [D, H], w1_d[l][:], f"w1_sb{l}") for l in range(DEPTH)]
        fb1_sb = [load(F32, [D, H // D], fb1_d[l][:], f"fb1_sb{l}")
                  for l in range(DEPTH)]
        w2_sb = [[load(F32, [D, D], w2_d[l][h * D:(h + 1) * D, :], f"w2_sb{l}_{h}")
                  for h in range(H // D)] for l in range(DEPTH)]
        bn_sb = {k: load(F32, [D, 1], v[:], f"bn_{k[0]}_{k[1]}")
                 for k, v in bn_d.items()}
        clsw_sb = load(F32, [D, 16], clsw_d[:], "clsw_sb")
        clsb_sb = load(F32, [16, 1], clsb_d[:], "clsb_sb")

        ident = wp.tile([128, 128], F32, name="ident")
        make_identity(nc, ident[:])
        ones_row = wp.tile([1, CHUNK], F32, name="ones_row")
        nc.vector.memset(ones_row[:], 1.0)

        # ---- persistent full-width activations (feature-major [D, sh])
        bufA = big.tile([D, sh], F32, name="bufA")  # agg / xp-src scratch
        bufB = big.tile([D, sh], F32, name="bufB")  # u(l0) -> xres2(l0)=xres(l1)
        bufC = big.tile([D, sh], F32, name="bufC")  # x0(l0) -> v(l1)
        bufD = big.tile([D, sh], F32, name="bufD")  # v(l0) -> u(l1)
        nc.sync.dma_start(out=bufC[:], in_=x0_d[:])

        def r32(ap):
            # float32r needs producer-side rounding (BIR verifier); plain
            # fp32 matmul for now. TODO: fp32r with rounded producers.
            return ap

        def bn_vec_math(sums_sb, g_sb, b_sb, a_out, c_out):
            """a = g*rsqrt(var+eps); c = b - mean*a, from [D,2] (sum, sumsq)."""
            m = sp.tile([D, 1], F32, tag="bnv", name="m")
            msq = sp.tile([D, 1], F32, tag="bnv", name="msq")
            var = sp.tile([D, 1], F32, tag="bnv", name="var")
            r = sp.tile([D, 1], F32, tag="bnv", name="r")
            nc.vector.tensor_scalar_mul(out=m[:], in0=sums_sb[:, 0:1],
                                        scalar1=1.0 / N)
            nc.vector.tensor_scalar_mul(out=msq[:], in0=sums_sb[:, 1:2],
                                        scalar1=1.0 / N)
            nc.vector.tensor_tensor(out=var[:], in0=m[:], in1=m[:], op=ALU.mult)
            nc.vector.tensor_tensor(out=var[:], in0=msq[:], in1=var[:],
                                    op=ALU.subtract)
            nc.vector.tensor_scalar_add(out=var[:], in0=var[:], scalar1=EPS)
            nc.vector.reciprocal(out=r[:], in_=var[:])
            nc.scalar.activation(out=a_out[:], in_=r[:], func=AF.Sqrt)
            nc.vector.tensor_tensor(out=a_out[:], in0=g_sb[:], in1=a_out[:],
                                    op=ALU.mult)
            nc.vector.tensor_tensor(out=c_out[:], in0=m[:], in1=a_out[:],
                                    op=ALU.mult)
            nc.vector.tensor_tensor(out=c_out[:], in0=b_sb[:], in1=c_out[:],
                                    op=ALU.subtract)

        def emit_stats_and_allreduce(src_buf, l, j, a_out, c_out):
            """Partial sum/sumsq of src_buf over real nodes -> AllReduce ->
            affine coefficients."""
            ssum = sp.tile([D, nch], F32, tag="stat", name=f"ssum{l}{j}")
            ssq = sp.tile([D, nch], F32, tag="stat", name=f"ssq{l}{j}")
            for ci, (c0, cw) in enumerate(chunks):
                rw = max(0, min(cw, sh_real - c0))
                if rw == 0:
                    nc.vector.memset(ssum[:, ci:ci + 1], 0.0)
                    nc.vector.memset(ssq[:, ci:ci + 1], 0.0)
                    continue
                sq = ck.tile([D, CHUNK], F32, tag="sq", name=f"sq{l}{j}{ci}")
                nc.vector.tensor_reduce(out=ssum[:, ci:ci + 1],
                                        in_=src_buf[:, c0:c0 + rw],
                                        axis=mybir.AxisListType.X, op=ALU.add)
                nc.vector.tensor_tensor(out=sq[:, :rw],
                                        in0=src_buf[:, c0:c0 + rw],
                                        in1=src_buf[:, c0:c0 + rw], op=ALU.mult)
                nc.vector.tensor_reduce(out=ssq[:, ci:ci + 1], in_=sq[:, :rw],
                                        axis=mybir.AxisListType.X, op=ALU.add)
            s2 = sp.tile([D, 2], F32, tag="s2", name=f"s2_{l}{j}")
            nc.vector.tensor_reduce(out=s2[:, 0:1], in_=ssum[:],
                                    axis=mybir.AxisListType.X, op=ALU.add)
            nc.vector.tensor_reduce(out=s2[:, 1:2], in_=ssq[:],
                                    axis=mybir.AxisListType.X, op=ALU.add)
            nc.sync.dma_start(out=bn_in[(l, j)][:], in_=s2[:])
            if dbg and l == 0 and j == 1:
                nc.sync.dma_start(out=dbg["dbg_s2"][:], in_=s2[:])
            nc.gpsimd.collective_compute(
                "AllReduce", ALU.add, replica_groups=rg,
                ins=[bn_in[(l, j)][:]], outs=[bn_out[(l, j)][:]])
            sums = sp.tile([D, 2], F32, tag="s2", name=f"sums{l}{j}")
            nc.sync.dma_start(out=sums[:], in_=bn_out[(l, j)][:])
            bn_vec_math(sums, bn_sb[(f"g{j}", l)], bn_sb[(f"b{j}", l)],
                        a_out, c_out)
            if dbg and l == 0 and j == 1:
                nc.sync.dma_start(out=dbg["dbg_sums"][:], in_=sums[:])
                nc.sync.dma_start(out=dbg["dbg_a1c1"][:, 0:1], in_=a_out[:])
                nc.sync.dma_start(out=dbg["dbg_a1c1"][:, 1:2], in_=c_out[:])

        for l in range(DEPTH):
            agg = bufA
            if l == 0:
                u, xres, v = bufB, bufC, bufD
            else:
                u, xres, v = bufD, bufB, bufC
            table = vtab

            # ---- aggregation: gather + tree reduce + invdeg + transpose
            for t in range(nt):
                K = K_t[t]
                G16 = gp.tile([128, kmax * D], F16, tag="G16", name=f"G{l}_{t}")
                if l == 0:
                    # layer-1 gather was expanded host-side; stream it
                    nc.sync.dma_start(
                        out=G16[:, :K * D],
                        in_=pay1_d[:, koff[t] * D:(koff[t] + K) * D])
                else:
                    for k in range(K):
                        nc.gpsimd.indirect_dma_start(
                            out=G16[:, k * D:(k + 1) * D], out_offset=None,
                            in_=table[:],
                            in_offset=IndirectOffsetOnAxis(
                                ap=gidx_sb[:, koff[t] + k:koff[t] + k + 1],
                                axis=0))
                G32 = gp.tile([128, (kmax // 2) * D], F32, tag="G32",
                              name=f"G32_{l}_{t}")
                acc = sp.tile([128, D], F32, tag="acc", name=f"acc{l}_{t}")
                _emit_tree_reduce(nc, G16, G32, K, acc)
                acc2 = sp.tile([128, D], F32, tag="acc2", name=f"acc2{l}_{t}")
                nc.vector.tensor_scalar_mul(out=acc2[:], in0=acc[:],
                                            scalar1=invdeg_sb[:, t:t + 1])
                ps = psB.tile([128, 128], F32, tag="tr", name=f"tr{l}_{t}")
                nc.tensor.transpose(ps[:], acc2[:], ident[:])
                nc.scalar.activation(out=agg[:, t * 128:(t + 1) * 128],
                                     in_=ps[:], func=AF.Copy)
            if dbg:
                nc.sync.dma_start(out=dbg["dbg_agg0" if l == 0 else "dbg_agg1"][:],
                                  in_=agg[:])

            # ---- dense sweep 1: GCN linear + residual -> u
            for c0, cw in chunks:
                sl = slice(c0, c0 + cw)
                ph = psA.tile([D, CHUNK], F32, tag="mm_gcn", name=f"ph{l}{c0}")
                nc.tensor.matmul(ph[:, :cw], r32(wg_sb[l][:]), r32(agg[:, sl]),
                                 start=True, stop=False)
                nc.tensor.matmul(ph[:, :cw], r32(bgT_sb[l][:]),
                                 r32(ones_row[:, :cw]), start=False, stop=True)
                nc.vector.tensor_tensor(out=u[:, sl], in0=ph[:, :cw],
                                        in1=xres[:, sl], op=ALU.add)

            if dbg and l == 0:
                nc.sync.dma_start(out=dbg["dbg_u0"][:], in_=u[:])
            a1 = sp.tile([D, 1], F32, tag="co", name=f"a1_{l}")
            c1 = sp.tile([D, 1], F32, tag="co", name=f"c1_{l}")
            emit_stats_and_allreduce(u, l, 1, a1, c1)

            # ---- dense sweep 2: BN1 affine -> FF -> v
            for c0, cw in chunks:
                sl = slice(c0, c0 + cw)
                xp = ck.tile([D, CHUNK], F32, tag="xp", name=f"xp{l}{c0}")
                nc.vector.tensor_scalar(out=xp[:, :cw], in0=u[:, sl],
                                        scalar1=a1[:], scalar2=c1[:],
                                        op0=ALU.mult, op1=ALU.add)
                py = psA.tile([D, CHUNK], F32, tag="mm_ff2", name=f"py{l}{c0}")
                for h in range(H // D):
                    pr = psA.tile([D, CHUNK], F32, tag=f"mm_ff1_{h}",
                                  name=f"pr{l}{c0}{h}")
                    nc.tensor.matmul(pr[:, :cw], r32(w1_sb[l][:, h * D:(h + 1) * D]),
                                     r32(xp[:, :cw]), start=True, stop=True)
                    rh = ck.tile([D, CHUNK], F32, tag=f"relu{h}",
                                 name=f"rh{l}{c0}{h}")
                    nc.scalar.activation(out=rh[:, :cw], in_=pr[:, :cw],
                                         func=AF.Relu, bias=fb1_sb[l][:, h:h + 1],
                                         scale=1.0)
                    nc.tensor.matmul(py[:, :cw], r32(w2_sb[l][h][:]),
                                     r32(rh[:, :cw]), start=(h == 0),
                                     stop=(h == H // D - 1))
                nc.vector.tensor_tensor(out=v[:, sl], in0=py[:, :cw],
                                        in1=xp[:, :cw], op=ALU.add)

            if dbg and l == 0:
                nc.sync.dma_start(out=dbg["dbg_v0"][:], in_=v[:])
            a2 = sp.tile([D, 1], F32, tag="co", name=f"a2_{l}")
            c2 = sp.tile([D, 1], F32, tag="co", name=f"c2_{l}")
            emit_stats_and_allreduce(v, l, 2, a2, c2)

            # ---- post-BN embeddings x'' (next residual / table / cls input)
            xnew = bufB if l == 0 else bufA
            for c0, cw in chunks:
                sl = slice(c0, c0 + cw)
                nc.vector.tensor_scalar(out=xnew[:, sl], in0=v[:, sl],
                                        scalar1=a2[:], scalar2=c2[:],
                                        op0=ALU.mult, op1=ALU.add)
            if sh > sh_real:
                nc.vector.memset(xnew[:, sh_real:sh], 0.0)

            if l == 0:
                # transpose to node-major, store shard, AllGather full table
                for t in range(nt):
                    ps = psB.tile([128, 128], F32, tag="tr", name=f"tv{t}")
                    nc.tensor.transpose(ps[:], xnew[:, t * 128:(t + 1) * 128],
                                        ident[:])
                    vT = sp.tile([128, D], F16, tag="vT", name=f"vT{t}")
                    nc.scalar.activation(out=vT[:], in_=ps[:], func=AF.Copy)
                    nc.sync.dma_start(out=vshard[t * 128:(t + 1) * 128, :],
                                      in_=vT[:])
                nc.gpsimd.collective_compute(
                    "AllGather", ALU.bypass, replica_groups=rg,
                    ins=[vshard[:]], outs=[vtab[:]])
                if dbg:
                    nc.sync.dma_start(out=dbg["dbg_xnew0"][:], in_=xnew[:])
                    nc.sync.dma_start(out=dbg["dbg_vtab"][:], in_=vtab[:])
            else:
                out_sb = wp.tile([16, sh], F32, name="out_sb")
                for c0, cw in chunks:
                    sl = slice(c0, c0 + cw)
                    pc = psA.tile([16, CHUNK], F32, tag="mm_gcn",
                                  name=f"pc{c0}")
                    nc.tensor.matmul(pc[:, :cw], r32(clsw_sb[:]),
                                     r32(xnew[:, sl]), start=True, stop=True)
                    nc.scalar.activation(out=out_sb[:, sl], in_=pc[:, :cw],
                                         func=AF.Identity, bias=clsb_sb[:],
                                         scale=1.0)
                nc.sync.dma_start(out=out_d[:], in_=out_sb[:])

    nc.compile()
    return nc


# ----------------------------------------------------------------------------
# Entry points
# ----------------------------------------------------------------------------

def _make_in_maps(cfg, inputs):
    W_gcn = np.asarray(inputs["W_gcn"], np.float32)
    b_gcn = np.asarray(inputs["b_gcn"], np.float32)
    ff_w1 = np.asarray(inputs["ff_w1"], np.float32)
    ff_b1 = np.asarray(inputs["ff_b1"], np.float32)
    ff_w2 = np.asarray(inputs["ff_w2"], np.float32)
    cls_w = np.asarray(inputs["cls_w"], np.float32)
    cls_b = np.asarray(inputs["cls_b"], np.float32)

    shared = {
        "clsw": np.ascontiguousarray(cls_w),
        "clsb": np.ascontiguousarray(cls_b.reshape(16, 1)),
    }
    for l in range(DEPTH):
        shared[f"wg{l}"] = np.ascontiguousarray(W_gcn[l])
        shared[f"bgT{l}"] = np.ascontiguousarray(b_gcn[l].reshape(1, D))
        shared[f"w1_{l}"] = np.ascontiguousarray(ff_w1[l])
        shared[f"fb1_{l}"] = np.ascontiguousarray(
            ff_b1[l].reshape(H // D, D).T)
        shared[f"w2_{l}"] = np.ascontiguousarray(ff_w2[l])
        shared[f"g1_{l}"] = np.ascontiguousarray(
            np.asarray(inputs["bn1_g"], np.float32)[l].reshape(D, 1))
        shared[f"b1_{l}"] = np.ascontiguousarray(
            np.asarray(inputs["bn1_b"], np.float32)[l].reshape(D, 1))
        shared[f"g2_{l}"] = np.ascontiguousarray(
            np.asarray(inputs["bn2_g"], np.float32)[l].reshape(D, 1))
        shared[f"b2_{l}"] = np.ascontiguousarray(
            np.asarray(inputs["bn2_b"], np.float32)[l].reshape(D, 1))

    sh = cfg["sh"]
    in_maps = []
    for c in range(CORES):
        m = dict(shared)
        m["x0_fm"] = np.ascontiguousarray(
            cfg["table0"][c * sh:(c + 1) * sh].T)
        m["pay1"] = cfg["pay1"][c]
        m["gidx"] = np.ascontiguousarray(cfg["gidx"][c])
        m["invdeg"] = np.ascontiguousarray(cfg["invdeg"][c])
        in_maps.append(m)
    return in_maps


def _postprocess(cfg, results):
    sh, sh_real = cfg["sh"], cfg["sh_real"]
    N = cfg["N"]
    node_of_tok = cfg["node_of_tok"]
    out = np.empty((N, 16), np.float32)
    for c in range(CORES):
        arr = results[c]["out_fm"]  # [16, sh]
        toks = np.arange(c * sh, c * sh + sh_real)
        out[node_of_tok[toks]] = arr.T[:sh_real]
    return out


def _ensure_axon_hooks():
    """The agent image's antenv lacks axon_hooks; synthesize it so
    bass_utils' trace=True path can find the NTFF profile hook."""
    try:
        import antenv.axon_hooks  # noqa: F401
        return
    except ImportError:
        pass
    import types
    import antenv
    mod = types.ModuleType("antenv.axon_hooks")
    mod._hook = None

    def set_axon_ntff_profile_hook(h):
        mod._hook = h

    def get_axon_ntff_profile_hook():
        return mod._hook

    mod.set_axon_ntff_profile_hook = set_axon_ntff_profile_hook
    mod.get_axon_ntff_profile_hook = get_axon_ntff_profile_hook
    sys.modules["antenv.axon_hooks"] = mod
    antenv.axon_hooks = mod
    try:
        from trn_agent_boot.trn_boot import _ntff_profile_via_ctypes
        h = _ntff_profile_via_ctypes("/opt/axon/libaxon_pjrt.so")
        if h is not None:
            mod._hook = h
    except Exception as e:  # pragma: no cover
        print(f"ntff hook setup failed: {e}", file=sys.stderr)


_CACHE = {}


def run(trace=False, **inputs):
    if trace:
        _ensure_axon_hooks()
    nodes = np.asarray(inputs["nodes"], np.float32)
    edge_src = np.asarray(inputs["edge_src"], np.int64)
    edge_dst = np.asarray(inputs["edge_dst"], np.int64)
    cfg = _prepare(nodes, edge_src, edge_dst)

    key = (nodes.shape, len(edge_src), tuple(cfg["K_t"]))
    if key not in _CACHE:
        _CACHE[key] = build_program(cfg)
    nc = _CACHE[key]

    in_maps = _make_in_maps(cfg, inputs)
    res = run_bass_kernel_spmd(nc, in_maps, list(range(CORES)), trace=trace)
    return _postprocess(cfg, res.results), res


def kernel(**inputs) -> np.ndarray:
    out, _ = run(trace=False, **inputs)
    return out



# revision 13
# speedup vs baseline: 1.2175x; 1.2175x over previous
"""Trainium2 Bass kernel for nn_NodeClassifier (gnn_message_passing).

Strategy (8 NeuronCores, SPMD):
  - Nodes block-partitioned by id across 8 cores (6250 each, padded to 6272
    slots; slot 0 and slots 6251..6271 are zero dummies), degree-sorted within
    each core so the padded neighbor grid (K-grid) is tight.
  - Layer-1 neighbor payload is host-expanded to fp16 and streamed.
  - Layer-2 gathers from the AllGathered fp16 embedding table via dma_gather
    (int16 indices). The 50176-row table is addressed through two window
    slices (rows 0.. and 17408..), each within int16 range; edges whose
    source falls in the overlap are assigned to windows per-node to balance
    the two K-grids (minimizes padding).
  - Neighbor reduction: two fp16 pairwise DVE passes (K -> K/4), then K/4
    accumulating PE matmuls  sum_k G_k^T @ diag(invdeg)  giving the
    feature-major aggregate in PSUM with transpose and 1/deg folded in.
  - Dense compute in bf16 on the PE (fp32 PSUM accumulation); BN statistics
    and residuals in fp32.
  - BN algebra folds: b_gcn cancels inside BN1; layer-1's BN2 affine is
    folded into layer-2's GCN weight (diag(a2) @ W; the c2 term cancels in
    the next BN1), so the AllGather ships raw pre-BN `v`; layer-2's BN2 is
    folded into the classifier weights.
  - BN statistics AllReduced ([D,2] per stage, tiny).
"""

import os
import sys
import numpy as np

for _p in ("/opt/trn_rl_repo",):
    if _p not in sys.path and os.path.isdir(_p):
        sys.path.insert(0, _p)

from contextlib import ExitStack

import concourse.bass as bass
import concourse.bacc as bacc
import concourse.mybir as mybir
import concourse.tile as tile
from concourse.bass_utils import run_bass_kernel_spmd
from concourse.masks import make_identity

F32 = mybir.dt.float32
F16 = mybir.dt.float16
BF16 = mybir.dt.bfloat16
I16 = mybir.dt.int16
AF = mybir.ActivationFunctionType
ALU = mybir.AluOpType

CORES = 8
D = 128
H = 512
DEPTH = 2
EPS = 1e-5
CHUNK = 512
WINB = 17408          # base row of table window B (window A base is 0)
WINSZ = 32768         # int16-addressable window size


def _bf16(a):
    import ml_dtypes
    return np.ascontiguousarray(np.asarray(a, np.float32).astype(ml_dtypes.bfloat16))


# ----------------------------------------------------------------------------
# Host-side preparation
# ----------------------------------------------------------------------------

def _prepare(nodes, edge_src, edge_dst):
    N = nodes.shape[0]
    assert N % CORES == 0
    sh_real = N // CORES
    nt = -(-(sh_real + 1) // 128)  # slot 0 reserved for a zero dummy
    sh = nt * 128
    assert sh - sh_real >= 2, "need zero dummies at both window pads"

    deg = np.bincount(edge_dst, minlength=N).astype(np.int64)
    assert deg.min() >= 1, "deg-0 nodes break the BN2 fold"

    # permutation: slot 0 dummy; real nodes at slots 1..sh_real, degree-sorted
    tok_of_node = np.empty(N, np.int64)
    node_of_tok = np.full(CORES * sh, -1, np.int64)
    for c in range(CORES):
        ids = np.arange(c * sh_real, (c + 1) * sh_real)
        order = np.argsort(deg[ids], kind="stable")
        toks = c * sh + 1 + np.arange(sh_real)
        tok_of_node[ids[order]] = toks
        node_of_tok[toks] = ids[order]

    tokv = CORES * sh
    assert tokv <= WINB + WINSZ, "table exceeds two int16 windows"
    assert WINB + WINSZ - tokv < WINB, "windows must overlap"

    # group edges by dst token
    dst_tok = tok_of_node[edge_dst]
    src_tok = tok_of_node[edge_src]
    order = np.argsort(dst_tok, kind="stable")
    dst_tok_s = dst_tok[order]
    src_tok_s = src_tok[order]
    cnt_tok = np.bincount(dst_tok_s, minlength=tokv)
    start_tok = np.concatenate([[0], np.cumsum(cnt_tok)[:-1]])

    # ---- L1 K-grid (single window; streamed payload), width multiple of 4
    cnt_mat = cnt_tok.reshape(CORES, nt, 128)
    K_t = np.maximum(cnt_mat.max(axis=(0, 2)), 4)
    K_t = K_t + ((-K_t) % 4)
    koff = np.concatenate([[0], np.cumsum(K_t)])
    ksum = int(koff[-1])

    # layer-1 payload values: fp16 x0 by token, zeros at dummies
    t16 = np.zeros((tokv, D), np.float16)
    real = node_of_tok >= 0
    t16[real] = nodes[node_of_tok[real]].astype(np.float16)

    e_slot = dst_tok_s % sh
    e_core = dst_tok_s // sh
    e_t = e_slot // 128
    e_r = np.arange(len(dst_tok_s)) - start_tok[dst_tok_s]

    gidx1 = np.zeros((CORES, 128, ksum), np.int64)  # token 0 = zero row
    gidx1[e_core, e_slot % 128, koff[e_t] + e_r] = src_tok_s
    pay1 = t16[gidx1]  # [CORES, 128, ksum, D]
    pay1 = np.ascontiguousarray(pay1.reshape(CORES, 128, ksum * D))

    # ---- L2 split K-grid over two table windows
    mustA_e = src_tok_s < WINB            # must use window A
    mustB_e = src_tok_s >= WINSZ          # must use window B
    cA = np.bincount(dst_tok_s[mustA_e], minlength=tokv).reshape(CORES, nt, 128)
    cB = np.bincount(dst_tok_s[mustB_e], minlength=tokv).reshape(CORES, nt, 128)
    maxA = cA.max(axis=(0, 2))
    maxB = cB.max(axis=(0, 2))
    need = np.maximum(K_t, maxA + maxB)
    need = need + (need % 2)
    K_A = maxA + (need - maxA - maxB + 1) // 2
    K_B = need - K_A
    koffA = np.concatenate([[0], np.cumsum(K_A)])
    koffB = np.concatenate([[0], np.cumsum(K_B)])
    ksumA, ksumB = int(koffA[-1]), int(koffB[-1])

    # per-node window assignment: nA = clamp(max(mustA, deg-K_B), <=K_A)
    cA_tok = cA.reshape(tokv)
    cB_tok = cB.reshape(tokv)
    KA_tok = K_A[(np.arange(tokv) % sh) // 128]
    KB_tok = K_B[(np.arange(tokv) % sh) // 128]
    nA_tok = np.maximum(cA_tok, cnt_tok - KB_tok)
    assert (nA_tok <= KA_tok).all() and (cnt_tok - nA_tok <= KB_tok).all()

    # order edges of each dst: mustA, then free, then mustB; first nA -> A
    sub = np.zeros(len(dst_tok_s), np.int8)
    sub[~mustA_e & ~mustB_e] = 1
    sub[mustB_e] = 2
    order2 = np.lexsort((sub, dst_tok_s))
    d2 = dst_tok_s[order2]
    s2 = src_tok_s[order2]
    r2 = np.arange(len(d2)) - start_tok[d2]
    toA = r2 < nA_tok[d2]
    # column index within the A / B grids
    colA = koffA[(d2 % sh) // 128] + r2
    rB = r2 - nA_tok[d2]
    colB = koffB[(d2 % sh) // 128] + rB

    # idx grids, int16, row = token - base; pads point at zero rows
    padA = 0                       # token 0 (dummy) in window A
    padB = tokv - 1 - WINB         # last dummy token in window B
    idxA = np.full((CORES, 128, ksumA), padA, np.int64)
    idxB = np.full((CORES, 128, ksumB), padB, np.int64)
    eA = toA
    idxA[d2[eA] // sh, (d2[eA] % sh) % 128, colA[eA]] = s2[eA]
    eB = ~toA
    idxB[d2[eB] // sh, (d2[eB] % sh) % 128, colB[eB]] = s2[eB] - WINB

    def pack_idx(grid, ncol):
        # [128 slots, ncols] -> dma_gather layout [128, ncols*8] int16:
        # per tile column k the 128 indices are at [(i%16), k*8 + i//16],
        # replicated into all 8 16-partition bands.
        out = np.empty((CORES, 128, ncol * 8), np.int16)
        for c in range(CORES):
            g = grid[c]  # [128, ncol]
            # i = slot p; idx i of column k at [i%16, 8k + i//16]
            a = g.T.reshape(ncol, 8, 16).transpose(2, 0, 1).reshape(16, ncol * 8)
            out[c] = np.tile(a.astype(np.int16), (8, 1))
        return out

    idxA16 = pack_idx(idxA, ksumA)
    idxB16 = pack_idx(idxB, ksumB)

    # per-core invdeg [128, nt] (0 for dummy slots)
    invdeg = np.zeros((CORES, 128, nt), np.float32)
    deg_tok = cnt_tok.reshape(CORES, sh)
    node_ok = (node_of_tok.reshape(CORES, sh) >= 0)
    iv = (1.0 / np.maximum(deg_tok, 1.0)) * node_ok
    for c in range(CORES):
        invdeg[c] = iv[c].reshape(nt, 128).T

    # per-core x0 feature-major [D, sh]
    x0_fm = np.zeros((CORES, D, sh), np.float32)
    for c in range(CORES):
        nm = node_of_tok[c * sh:(c + 1) * sh]
        ok = nm >= 0
        blk = np.zeros((sh, D), np.float32)
        blk[ok] = nodes[nm[ok]]
        x0_fm[c] = blk.T

    return dict(
        N=N, sh_real=sh_real, sh=sh, nt=nt, tokv=tokv,
        K_t=[int(k) for k in K_t], koff=[int(k) for k in koff], ksum=ksum,
        K_A=[int(k) for k in K_A], koffA=[int(k) for k in koffA], ksumA=ksumA,
        K_B=[int(k) for k in K_B], koffB=[int(k) for k in koffB], ksumB=ksumB,
        idxA16=idxA16, idxB16=idxB16, invdeg=invdeg, pay1=pay1, x0_fm=x0_fm,
        node_of_tok=node_of_tok,
    )


# ----------------------------------------------------------------------------
# Program builder
# ----------------------------------------------------------------------------

def build_program(cfg, debug=False):
    nt, sh, sh_real = cfg["nt"], cfg["sh"], cfg["sh_real"]
    tokv = cfg["tokv"]
    K_t, koff, ksum = cfg["K_t"], cfg["koff"], cfg["ksum"]
    K_A, koffA, ksumA = cfg["K_A"], cfg["koffA"], cfg["ksumA"]
    K_B, koffB, ksumB = cfg["K_B"], cfg["koffB"], cfg["ksumB"]
    N = cfg["N"]
    kmax = max(max(K_t), max(a + b for a, b in zip(K_A, K_B)))
    rg = [list(range(CORES))]

    chunks = []
    c0 = 0
    while c0 < sh:
        cw = min(CHUNK, sh - c0)
        chunks.append((c0, cw))
        c0 += cw
    nch = len(chunks)

    nc = bacc.Bacc("TRN2", target_bir_lowering=False, debug=False,
                   num_devices=CORES)

    # ---- I/O declarations
    pay1_d = nc.dram_tensor("pay1", [128, ksum * D], F16, kind="ExternalInput")
    x0_d = nc.dram_tensor("x0_fm", [D, sh], F32, kind="ExternalInput")
    idxA_d = nc.dram_tensor("idxA", [128, ksumA * 8], I16, kind="ExternalInput")
    idxB_d = nc.dram_tensor("idxB", [128, ksumB * 8], I16, kind="ExternalInput")
    invdeg_d = nc.dram_tensor("invdeg", [128, nt], F32, kind="ExternalInput")
    wg_d = [nc.dram_tensor(f"wg{l}", [D, D], BF16, kind="ExternalInput")
            for l in range(DEPTH)]
    w1_d = [nc.dram_tensor(f"w1_{l}", [D, H], BF16, kind="ExternalInput")
            for l in range(DEPTH)]
    fb1_d = [nc.dram_tensor(f"fb1_{l}", [D, H // D], F32, kind="ExternalInput")
             for l in range(DEPTH)]
    w2_d = [nc.dram_tensor(f"w2_{l}", [H, D], BF16, kind="ExternalInput")
            for l in range(DEPTH)]
    bn_d = {}
    for l in range(DEPTH):
        for nm in ("g1", "b1", "g2", "b2"):
            bn_d[(nm, l)] = nc.dram_tensor(f"{nm}_{l}", [D, 1], F32,
                                           kind="ExternalInput")
    clsw_d = nc.dram_tensor("clsw", [D, 16], F32, kind="ExternalInput")
    clsb_d = nc.dram_tensor("clsb", [16, 1], F32, kind="ExternalInput")
    out_d = nc.dram_tensor("out_fm", [16, sh], F32, kind="ExternalOutput")
    dbg = {}
    if debug:
        for nm, shape, dt_ in [("dbg_u0", [D, sh], F32),
                               ("dbg_v0", [D, sh], F32),
                               ("dbg_u1", [D, sh], F32),
                               ("dbg_vtab", [tokv, D], F16),
                               ("dbg_g16a", [128, kmax * D], F16),
                               ("dbg_g16b", [128, kmax * D], F16)]:
            dbg[nm] = nc.dram_tensor(nm, shape, dt_, kind="ExternalOutput")

    with tile.TileContext(nc) as tc, ExitStack() as ctx:
        dram = ctx.enter_context(tc.tile_pool(name="dram", bufs=1, space="DRAM"))
        wp = ctx.enter_context(tc.tile_pool(name="weights", bufs=1))
        big = ctx.enter_context(tc.tile_pool(name="big", bufs=1))
        gp = ctx.enter_context(tc.tile_pool(name="gather", bufs=2))
        g8p = ctx.enter_context(tc.tile_pool(name="g8", bufs=2))
        ck3 = ctx.enter_context(tc.tile_pool(name="aggc", bufs=3))
        ckx = ctx.enter_context(tc.tile_pool(name="ckx", bufs=2))
        sp = ctx.enter_context(tc.tile_pool(name="small", bufs=4))
        psR = ctx.enter_context(tc.tile_pool(name="psR", bufs=2, space="PSUM"))
        psM = ctx.enter_context(tc.tile_pool(name="psM", bufs=2, space="PSUM"))
        psP = ctx.enter_context(tc.tile_pool(name="psP", bufs=2, space="PSUM"))
        psY = ctx.enter_context(tc.tile_pool(name="psY", bufs=2, space="PSUM"))

        # ---- internal DRAM (collective bounce buffers)
        vshard = dram.tile([sh, D], F16, name="vshard")
        vtab = dram.tile([tokv, D], F16, addr_space="Shared", name="vtab")
        bn_in, bn_out = {}, {}
        for l in range(DEPTH):
            for j in (1, 2):
                bn_in[(l, j)] = dram.tile([D, 2], F32, name=f"bni{l}{j}")
                bn_out[(l, j)] = dram.tile([D, 2], F32, addr_space="Shared",
                                           name=f"bno{l}{j}")

        # ---- load constants / weights to SBUF (scalar = HWDGE ring B,
        # leaving the sync ring free for the pay1 stream)
        def load(dt_, shape, src, name):
            t = wp.tile(shape, dt_, name=name)
            nc.scalar.dma_start(out=t[:], in_=src)
            return t

        idxA_sb = load(I16, [128, ksumA * 8], idxA_d[:], "idxA_sb")
        idxB_sb = load(I16, [128, ksumB * 8], idxB_d[:], "idxB_sb")
        invdeg_sb = load(F32, [128, nt], invdeg_d[:], "invdeg_sb")
        wg_sb = [load(BF16, [D, D], wg_d[l][:], f"wg_sb{l}")
                 for l in range(DEPTH)]
        w1_sb = [load(BF16, [D, H], w1_d[l][:], f"w1_sb{l}")
                 for l in range(DEPTH)]
        fb1_sb = [load(F32, [D, H // D], fb1_d[l][:], f"fb1_sb{l}")
                  for l in range(DEPTH)]
        w2_sb = [[load(BF16, [D, D], w2_d[l][h * D:(h + 1) * D, :],
                       f"w2_sb{l}_{h}") for h in range(H // D)]
                 for l in range(DEPTH)]
        bn_sb = {k: load(F32, [D, 1], v[:], f"bn_{k[0]}_{k[1]}")
                 for k, v in bn_d.items()}
        clsw_sb = load(F32, [D, 16], clsw_d[:], "clsw_sb")
        clsb_sb = load(F32, [16, 1], clsb_d[:], "clsb_sb")
        wg1p_sb = wp.tile([D, D], BF16, name="wg1p")     # diag(a2) @ W_gcn[1]
        clswp_sb = wp.tile([D, 16], BF16, name="clswp")  # diag(a2') @ cls_w
        biasF_sb = wp.tile([16, 1], F32, name="biasF")   # c2' @ cls_w + cls_b

        ident16 = wp.tile([128, 128], F16, name="ident16")
        make_identity(nc, ident16[:])
        ident32 = wp.tile([128, 128], F32, name="ident32")
        make_identity(nc, ident32[:])

        # diag(invdeg) per tile, fp16
        diag_all = wp.tile([128, nt * 128], F16, name="diag_all")
        for t in range(nt):
            nc.vector.tensor_scalar_mul(
                out=diag_all[:, t * 128:(t + 1) * 128], in0=ident16[:],
                scalar1=invdeg_sb[:, t:t + 1])

        # ---- persistent activations (feature-major [D, sh] fp32)
        xres = big.tile([D, sh], F32, name="xres")  # x0, then xnew(l0)
        ubuf = big.tile([D, sh], F32, name="ubuf")
        vbuf = big.tile([D, sh], F32, name="vbuf")
        nc.scalar.dma_start(out=xres[:], in_=x0_d[:])

        def bn_vec_math(sums_sb, g_sb, b_sb, a_out, c_out, tag):
            """a = g*rsqrt(var+eps); c = b - mean*a, from [D,2] (sum, sumsq)."""
            m = sp.tile([D, 1], F32, tag="bnv", name=f"m{tag}")
            msq = sp.tile([D, 1], F32, tag="bnv", name=f"msq{tag}")
            var = sp.tile([D, 1], F32, tag="bnv", name=f"var{tag}")
            r = sp.tile([D, 1], F32, tag="bnv", name=f"r{tag}")
            nc.vector.tensor_scalar_mul(out=m[:], in0=sums_sb[:, 0:1],
                                        scalar1=1.0 / N)
            nc.vector.tensor_scalar_mul(out=msq[:], in0=sums_sb[:, 1:2],
                                        scalar1=1.0 / N)
            nc.vector.tensor_tensor(out=var[:], in0=m[:], in1=m[:], op=ALU.mult)
            nc.vector.tensor_tensor(out=var[:], in0=msq[:], in1=var[:],
                                    op=ALU.subtract)
            nc.vector.tensor_scalar_add(out=var[:], in0=var[:], scalar1=EPS)
            nc.vector.reciprocal(out=r[:], in_=var[:])
            nc.scalar.activation(out=a_out[:], in_=r[:], func=AF.Sqrt)
            nc.vector.tensor_tensor(out=a_out[:], in0=g_sb[:], in1=a_out[:],
                                    op=ALU.mult)
            nc.vector.tensor_tensor(out=c_out[:], in0=m[:], in1=a_out[:],
                                    op=ALU.mult)
            nc.vector.tensor_tensor(out=c_out[:], in0=b_sb[:], in1=c_out[:],
                                    op=ALU.subtract)

        def allreduce_stats(ssum, ssq, l, j, a_out, c_out):
            s2 = sp.tile([D, 2], F32, tag="s2", name=f"s2_{l}{j}")
            nc.vector.tensor_reduce(out=s2[:, 0:1], in_=ssum[:],
                                    axis=mybir.AxisListType.X, op=ALU.add)
            nc.vector.tensor_reduce(out=s2[:, 1:2], in_=ssq[:],
                                    axis=mybir.AxisListType.X, op=ALU.add)
            nc.sync.dma_start(out=bn_in[(l, j)][:], in_=s2[:])
            nc.gpsimd.collective_compute(
                "AllReduce", ALU.add, replica_groups=rg,
                ins=[bn_in[(l, j)][:]], outs=[bn_out[(l, j)][:]])
            sums = sp.tile([D, 2], F32, tag="s2", name=f"sums{l}{j}")
            nc.sync.dma_start(out=sums[:], in_=bn_out[(l, j)][:])
            bn_vec_math(sums, bn_sb[(f"g{j}", l)], bn_sb[(f"b{j}", l)],
                        a_out, c_out, f"{l}{j}")

        def emit_gather(l, t, G16):
            """Fill G16[:, :W*D] with the tile's neighbor rows (fp16)."""
            if l == 0:
                K = K_t[t]
                nc.sync.dma_start(
                    out=G16[:, :K * D],
                    in_=pay1_d[:, koff[t] * D:(koff[t] + K) * D])
                return K
            KA, KB = K_A[t], K_B[t]
            if KA:
                nc.gpsimd.dma_gather(
                    out_ap=G16[:, :KA * D].rearrange(
                        "p (g d) -> p g d", g=KA, d=D),
                    in_ap=vtab[0:WINSZ, :],
                    idxs_ap=idxA_sb[:, koffA[t] * 8:(koffA[t] + KA) * 8],
                    num_idxs=KA * 128, num_idxs_reg=KA * 128, elem_size=D,
                    single_packet=False)
            if KB:
                nc.gpsimd.dma_gather(
                    out_ap=G16[:, KA * D:(KA + KB) * D].rearrange(
                        "p (g d) -> p g d", g=KB, d=D),
                    in_ap=vtab[WINB:WINB + WINSZ, :],
                    idxs_ap=idxB_sb[:, koffB[t] * 8:(koffB[t] + KB) * 8],
                    num_idxs=KB * 128, num_idxs_reg=KB * 128, elem_size=D,
                    single_packet=False)
            return KA + KB

        for l in range(DEPTH):
            # ---------- aggregation + GCN sweep, pipelined per chunk
            ssum1 = sp.tile([D, nch], F32, tag="st1", name=f"ssum1_{l}")
            ssq1 = sp.tile([D, nch], F32, tag="st1", name=f"ssq1_{l}")
            for ci, (c0, cw) in enumerate(chunks):
                tiles = range(c0 // 128, (c0 + cw) // 128)
                aggc = ck3.tile([D, CHUNK], BF16, tag="aggc",
                                name=f"aggc{l}_{ci}")
                for t in tiles:
                    G16 = gp.tile([128, kmax * D], F16, tag="G16",
                                  name=f"G{l}_{t}")
                    W = emit_gather(l, t, G16)
                    if dbg and l == 1 and t in (5, 30):
                        nc.sync.dma_start(
                            out=dbg["dbg_g16a" if t == 5 else "dbg_g16b"][:, :W * D],
                            in_=G16[:, :W * D])
                    # fp16 pairwise passes: W -> W/2 (-> W/4 when W%4==0)
                    h2 = W // 2
                    G8 = g8p.tile([128, (kmax // 2) * D], F16, tag="G8",
                                  name=f"G8_{l}_{t}")
                    nc.vector.tensor_tensor(
                        out=G8[:, :h2 * D], in0=G16[:, :h2 * D],
                        in1=G16[:, h2 * D:W * D], op=ALU.add)
                    if h2 % 2 == 0:
                        h4 = h2 // 2
                        G4 = g8p.tile([128, (kmax // 4) * D], F16, tag="G4",
                                      name=f"G4_{l}_{t}")
                        nc.vector.tensor_tensor(
                            out=G4[:, :h4 * D], in0=G8[:, :h4 * D],
                            in1=G8[:, h4 * D:h2 * D], op=ALU.add)
                        red_src, nred = G4, h4
                    else:
                        red_src, nred = G8, h2
                    ps = psR.tile([D, 128], F32, tag="red", name=f"red{l}_{t}")
                    dg = diag_all[:, t * 128:(t + 1) * 128]
                    for k in range(nred):
                        nc.tensor.matmul(ps[:], red_src[:, k * D:(k + 1) * D],
                                         dg, start=(k == 0),
                                         stop=(k == nred - 1))
                    nc.scalar.activation(
                        out=aggc[:, (t - c0 // 128) * 128:
                                 (t - c0 // 128 + 1) * 128],
                        in_=ps[:], func=AF.Copy)
                # GCN linear + residual -> u
                sl = slice(c0, c0 + cw)
                wgl = wg_sb[0] if l == 0 else wg1p_sb
                ph = psM.tile([D, CHUNK], F32, tag="mm1", name=f"ph{l}{c0}")
                nc.tensor.matmul(ph[:, :cw], wgl[:], aggc[:, :cw],
                                 start=True, stop=True)
                nc.vector.tensor_tensor(out=ubuf[:, sl], in0=ph[:, :cw],
                                        in1=xres[:, sl], op=ALU.add)
                # BN1 partial stats over real columns (slot 0 is a dummy but
                # u(dummy) == 0 so including it is harmless)
                rw = max(0, min(cw, sh_real + 1 - c0))
                if rw == 0:
                    nc.vector.memset(ssum1[:, ci:ci + 1], 0.0)
                    nc.vector.memset(ssq1[:, ci:ci + 1], 0.0)
                else:
                    nc.vector.tensor_reduce(out=ssum1[:, ci:ci + 1],
                                            in_=ubuf[:, c0:c0 + rw],
                                            axis=mybir.AxisListType.X,
                                            op=ALU.add)
                    sqs = ckx.tile([D, CHUNK], F32, tag="sqs",
                                   name=f"sq1{l}{ci}")
                    nc.scalar.activation(out=sqs[:, :rw],
                                         in_=ubuf[:, c0:c0 + rw],
                                         func=AF.Square,
                                         accum_out=ssq1[:, ci:ci + 1])
            if dbg and l == 0:
                nc.sync.dma_start(out=dbg["dbg_u0"][:], in_=ubuf[:])
            if dbg and l == 1:
                nc.sync.dma_start(out=dbg["dbg_u1"][:], in_=ubuf[:])
            a1 = sp.tile([D, 1], F32, tag="co", name=f"a1_{l}")
            c1 = sp.tile([D, 1], F32, tag="co", name=f"c1_{l}")
            allreduce_stats(ssum1, ssq1, l, 1, a1, c1)

            # ---------- FF sweep (+ layer-0: v transposes & AllGather)
            ssum2 = sp.tile([D, nch], F32, tag="st2", name=f"ssum2_{l}")
            ssq2 = sp.tile([D, nch], F32, tag="st2", name=f"ssq2_{l}")
            next_tp = 0
            for ci, (c0, cw) in enumerate(chunks):
                sl = slice(c0, c0 + cw)
                xp = ckx.tile([D, CHUNK], F32, tag="xp", name=f"xp{l}{c0}")
                nc.vector.tensor_scalar(out=xp[:, :cw], in0=ubuf[:, sl],
                                        scalar1=a1[:], scalar2=c1[:],
                                        op0=ALU.mult, op1=ALU.add)
                xpb = ckx.tile([D, CHUNK], BF16, tag="xpb", name=f"xb{l}{c0}")
                nc.scalar.activation(out=xpb[:, :cw], in_=ubuf[:, sl],
                                     func=AF.Identity, bias=c1[:],
                                     scale=a1[:])
                py = psY.tile([D, CHUNK], F32, tag="py", name=f"py{l}{c0}")
                for h in range(H // D):
                    pr = psP.tile([128, CHUNK], F32, tag="pr",
                                  name=f"pr{l}{c0}{h}")
                    nc.tensor.matmul(pr[:, :cw],
                                     w1_sb[l][:, h * D:(h + 1) * D],
                                     xpb[:, :cw], start=True, stop=True)
                    rh = ckx.tile([128, CHUNK], BF16, tag="rh",
                                  name=f"rh{l}{c0}{h}")
                    nc.scalar.activation(out=rh[:, :cw], in_=pr[:, :cw],
                                         func=AF.Relu,
                                         bias=fb1_sb[l][:, h:h + 1], scale=1.0)
                    nc.tensor.matmul(py[:, :cw], w2_sb[l][h][:], rh[:, :cw],
                                     start=(h == 0), stop=(h == H // D - 1))
                nc.vector.tensor_tensor(out=vbuf[:, sl], in0=py[:, :cw],
                                        in1=xp[:, :cw], op=ALU.add)
                # BN2 partial stats (DVE); must exclude dummies: v(dummy) != 0
                lo = c0 + 1 if c0 == 0 else c0  # skip slot 0
                hi = min(c0 + cw, sh_real + 1)
                if hi <= lo:
                    nc.vector.memset(ssum2[:, ci:ci + 1], 0.0)
                    nc.vector.memset(ssq2[:, ci:ci + 1], 0.0)
                else:
                    nc.vector.tensor_reduce(out=ssum2[:, ci:ci + 1],
                                            in_=vbuf[:, lo:hi],
                                            axis=mybir.AxisListType.X,
                                            op=ALU.add)
                    sqv = ckx.tile([D, CHUNK], F32, tag="sqs",
                                   name=f"sq2{l}{ci}")
                    nc.vector.tensor_tensor(out=sqv[:, :hi - lo],
                                            in0=vbuf[:, lo:hi],
                                            in1=vbuf[:, lo:hi],
                                            op=ALU.mult)
                    nc.vector.tensor_reduce(out=ssq2[:, ci:ci + 1],
                                            in_=sqv[:, :hi - lo],
                                            axis=mybir.AxisListType.X,
                                            op=ALU.add)
                if l == 0:
                    # zero dummy columns before tiles are shipped
                    if ci == 0:
                        nc.vector.memset(vbuf[:, 0:1], 0.0)
                    if ci == nch - 1 and sh > sh_real + 1:
                        nc.vector.memset(vbuf[:, sh_real + 1:sh], 0.0)
                    done_t = (c0 + cw) // 128
                    while next_tp < done_t:
                        t = next_tp
                        pt = psR.tile([128, 128], F32, tag="red",
                                      name=f"vt{t}")
                        nc.tensor.transpose(pt[:],
                                            vbuf[:, t * 128:(t + 1) * 128],
                                            ident32[:])
                        vT = sp.tile([128, D], F16, tag="vT", name=f"vT{t}")
                        nc.scalar.activation(out=vT[:], in_=pt[:], func=AF.Copy)
                        nc.scalar.dma_start(
                            out=vshard[t * 128:(t + 1) * 128, :], in_=vT[:])
                        next_tp += 1
                    if next_tp == nt:
                        nc.gpsimd.collective_compute(
                            "AllGather", ALU.bypass, replica_groups=rg,
                            ins=[vshard[:]], outs=[vtab[:]])
                        next_tp += 1
            if dbg and l == 0:
                nc.sync.dma_start(out=dbg["dbg_v0"][:], in_=vbuf[:])
                nc.sync.dma_start(out=dbg["dbg_vtab"][:], in_=vtab[:])
            a2 = sp.tile([D, 1], F32, tag="co", name=f"a2_{l}")
            c2 = sp.tile([D, 1], F32, tag="co", name=f"c2_{l}")
            allreduce_stats(ssum2, ssq2, l, 2, a2, c2)

            if l == 0:
                # fold BN2 affine into layer-2 GCN weight; local xnew residual
                nc.vector.tensor_scalar_mul(out=wg1p_sb[:], in0=wg_sb[1][:],
                                            scalar1=a2[:])
                for c0, cw in chunks:
                    sl = slice(c0, c0 + cw)
                    nc.vector.tensor_scalar(out=xres[:, sl], in0=vbuf[:, sl],
                                            scalar1=a2[:], scalar2=c2[:],
                                            op0=ALU.mult, op1=ALU.add)
            else:
                # classifier with BN2 folded in:
                # logits = v2 @ (diag(a2) clsw) + (c2 @ clsw + clsb)
                nc.vector.tensor_scalar_mul(out=clswp_sb[:], in0=clsw_sb[:],
                                            scalar1=a2[:])
                pb = psM.tile([16, CHUNK], F32, tag="mm1", name="pb")
                nc.tensor.matmul(pb[:, 0:1], clsw_sb[:], c2[:],
                                 start=True, stop=True)
                nc.vector.tensor_tensor(out=biasF_sb[:], in0=pb[:, 0:1],
                                        in1=clsb_sb[:], op=ALU.add)
                out_sb = wp.tile([16, sh], F32, name="out_sb")
                for c0, cw in chunks:
                    sl = slice(c0, c0 + cw)
                    vb = ckx.tile([D, CHUNK], BF16, tag="vb", name=f"vb{c0}")
                    nc.scalar.activation(out=vb[:, :cw], in_=vbuf[:, sl],
                                         func=AF.Copy)
                    pc = psM.tile([16, CHUNK], F32, tag="mm1", name=f"pc{c0}")
                    nc.tensor.matmul(pc[:, :cw], clswp_sb[:], vb[:, :cw],
                                     start=True, stop=True)
                    nc.scalar.activation(out=out_sb[:, sl], in_=pc[:, :cw],
                                         func=AF.Identity, bias=biasF_sb[:],
                                         scale=1.0)
                    nc.sync.dma_start(out=out_d[:, sl], in_=out_sb[:, sl])

    nc.compile()
    return nc


# ----------------------------------------------------------------------------
# Entry points
# ----------------------------------------------------------------------------

def _make_in_maps(cfg, inputs):
    W_gcn = np.asarray(inputs["W_gcn"], np.float32)
    ff_w1 = np.asarray(inputs["ff_w1"], np.float32)
    ff_b1 = np.asarray(inputs["ff_b1"], np.float32)
    ff_w2 = np.asarray(inputs["ff_w2"], np.float32)
    cls_w = np.asarray(inputs["cls_w"], np.float32)
    cls_b = np.asarray(inputs["cls_b"], np.float32)

    shared = {
        "clsw": np.ascontiguousarray(cls_w),
        "clsb": np.ascontiguousarray(cls_b.reshape(16, 1)),
    }
    for l in range(DEPTH):
        shared[f"wg{l}"] = _bf16(W_gcn[l])
        shared[f"w1_{l}"] = _bf16(ff_w1[l])
        shared[f"fb1_{l}"] = np.ascontiguousarray(
            ff_b1[l].reshape(H // D, D).T)
        shared[f"w2_{l}"] = _bf16(ff_w2[l])
        shared[f"g1_{l}"] = np.ascontiguousarray(
            np.asarray(inputs["bn1_g"], np.float32)[l].reshape(D, 1))
        shared[f"b1_{l}"] = np.ascontiguousarray(
            np.asarray(inputs["bn1_b"], np.float32)[l].reshape(D, 1))
        shared[f"g2_{l}"] = np.ascontiguousarray(
            np.asarray(inputs["bn2_g"], np.float32)[l].reshape(D, 1))
        shared[f"b2_{l}"] = np.ascontiguousarray(
            np.asarray(inputs["bn2_b"], np.float32)[l].reshape(D, 1))

    in_maps = []
    for c in range(CORES):
        m = dict(shared)
        m["x0_fm"] = np.ascontiguousarray(cfg["x0_fm"][c])
        m["pay1"] = cfg["pay1"][c]
        m["idxA"] = np.ascontiguousarray(cfg["idxA16"][c])
        m["idxB"] = np.ascontiguousarray(cfg["idxB16"][c])
        m["invdeg"] = np.ascontiguousarray(cfg["invdeg"][c])
        in_maps.append(m)
    return in_maps


def _postprocess(cfg, results):
    sh = cfg["sh"]
    N = cfg["N"]
    node_of_tok = cfg["node_of_tok"]
    out = np.empty((N, 16), np.float32)
    for c in range(CORES):
        arr = results[c]["out_fm"]  # [16, sh]
        toks = np.arange(c * sh, (c + 1) * sh)
        sel = node_of_tok[toks] >= 0
        out[node_of_tok[toks[sel]]] = arr.T[sel]
    return out


def _ensure_axon_hooks():
    """The agent image's antenv lacks axon_hooks; synthesize it so
    bass_utils' trace=True path can find the NTFF profile hook."""
    try:
        import antenv.axon_hooks  # noqa: F401
        return
    except ImportError:
        pass
    import types
    import antenv
    mod = types.ModuleType("antenv.axon_hooks")
    mod._hook = None

    def set_axon_ntff_profile_hook(h):
        mod._hook = h

    def get_axon_ntff_profile_hook():
        return mod._hook

    mod.set_axon_ntff_profile_hook = set_axon_ntff_profile_hook
    mod.get_axon_ntff_profile_hook = get_axon_ntff_profile_hook
    sys.modules["antenv.axon_hooks"] = mod
    antenv.axon_hooks = mod
    try:
        from trn_agent_boot.trn_boot import _ntff_profile_via_ctypes
        h = _ntff_profile_via_ctypes("/opt/axon/libaxon_pjrt.so")
        if h is not None:
            mod._hook = h
    except Exception as e:  # pragma: no cover
        print(f"ntff hook setup failed: {e}", file=sys.stderr)


_CACHE = {}


def run(trace=False, debug=False, **inputs):
    if trace:
        _ensure_axon_hooks()
    nodes = np.asarray(inputs["nodes"], np.float32)
    edge_src = np.asarray(inputs["edge_src"], np.int64)
    edge_dst = np.asarray(inputs["edge_dst"], np.int64)
    cfg = _prepare(nodes, edge_src, edge_dst)

    key = (nodes.shape, len(edge_src), tuple(cfg["K_t"]), debug)
    if key not in _CACHE:
        _CACHE[key] = build_program(cfg, debug=debug)
    nc = _CACHE[key]

    in_maps = _make_in_maps(cfg, inputs)
    res = run_bass_kernel_spmd(nc, in_maps, list(range(CORES)), trace=trace)
    return _postprocess(cfg, res.results), res


def kernel(**inputs) -> np.ndarray:
    out, _ = run(trace=False, **inputs)
    return out


# revision 14
# speedup vs baseline: 1.2269x; 1.0077x over previous
"""Trainium2 Bass kernel for nn_NodeClassifier (gnn_message_passing).

Strategy (8 NeuronCores, SPMD):
  - Nodes block-partitioned by id across 8 cores (6250 each, padded to 6272
    slots; slot 0 and slots 6251..6271 are zero dummies), degree-sorted within
    each core so the padded neighbor grid (K-grid) is tight.
  - Layer-1 neighbor payload is host-expanded to fp16 and streamed.
  - Layer-2 gathers from the AllGathered fp16 embedding table via dma_gather
    (int16 indices). The 50176-row table is addressed through two window
    slices (rows 0.. and 17408..), each within int16 range; edges whose
    source falls in the overlap are assigned to windows per-node to balance
    the two K-grids (minimizes padding).
  - Neighbor reduction: two fp16 pairwise DVE passes (K -> K/4), then K/4
    accumulating PE matmuls  sum_k G_k^T @ diag(invdeg)  giving the
    feature-major aggregate in PSUM with transpose and 1/deg folded in.
  - Dense compute in bf16 on the PE (fp32 PSUM accumulation); BN statistics
    and residuals in fp32.
  - BN algebra folds: b_gcn cancels inside BN1; layer-1's BN2 affine is
    folded into layer-2's GCN weight (diag(a2) @ W; the c2 term cancels in
    the next BN1), so the AllGather ships raw pre-BN `v`; layer-2's BN2 is
    folded into the classifier weights.
  - BN statistics AllReduced ([D,2] per stage, tiny).
"""

import os
import sys
import numpy as np

for _p in ("/opt/trn_rl_repo",):
    if _p not in sys.path and os.path.isdir(_p):
        sys.path.insert(0, _p)

from contextlib import ExitStack

import concourse.bass as bass
import concourse.bacc as bacc
import concourse.mybir as mybir
import concourse.tile as tile
from concourse.bass_utils import run_bass_kernel_spmd
from concourse.masks import make_identity

F32 = mybir.dt.float32
F16 = mybir.dt.float16
BF16 = mybir.dt.bfloat16
I16 = mybir.dt.int16
AF = mybir.ActivationFunctionType
ALU = mybir.AluOpType

CORES = 8
D = 128
H = 512
DEPTH = 2
EPS = 1e-5
CHUNK = 512
WINB = 17408          # base row of table window B (window A base is 0)
WINSZ = 32768         # int16-addressable window size


def _bf16(a):
    import ml_dtypes
    return np.ascontiguousarray(np.asarray(a, np.float32).astype(ml_dtypes.bfloat16))


# ----------------------------------------------------------------------------
# Host-side preparation
# ----------------------------------------------------------------------------

def _prepare(nodes, edge_src, edge_dst):
    N = nodes.shape[0]
    assert N % CORES == 0
    sh_real = N // CORES
    nt = -(-(sh_real + 1) // 128)  # slot 0 reserved for a zero dummy
    sh = nt * 128
    assert sh - sh_real >= 2, "need zero dummies at both window pads"

    deg = np.bincount(edge_dst, minlength=N).astype(np.int64)
    assert deg.min() >= 1, "deg-0 nodes break the BN2 fold"

    # permutation: slot 0 dummy; real nodes at slots 1..sh_real, degree-sorted
    tok_of_node = np.empty(N, np.int64)
    node_of_tok = np.full(CORES * sh, -1, np.int64)
    for c in range(CORES):
        ids = np.arange(c * sh_real, (c + 1) * sh_real)
        order = np.argsort(deg[ids], kind="stable")
        toks = c * sh + 1 + np.arange(sh_real)
        tok_of_node[ids[order]] = toks
        node_of_tok[toks] = ids[order]

    tokv = CORES * sh
    assert tokv <= WINB + WINSZ, "table exceeds two int16 windows"
    assert WINB + WINSZ - tokv < WINB, "windows must overlap"

    # group edges by dst token
    dst_tok = tok_of_node[edge_dst]
    src_tok = tok_of_node[edge_src]
    order = np.argsort(dst_tok, kind="stable")
    dst_tok_s = dst_tok[order]
    src_tok_s = src_tok[order]
    cnt_tok = np.bincount(dst_tok_s, minlength=tokv)
    start_tok = np.concatenate([[0], np.cumsum(cnt_tok)[:-1]])

    # ---- L1 K-grid (single window; streamed payload), width multiple of 4
    cnt_mat = cnt_tok.reshape(CORES, nt, 128)
    K_t = np.maximum(cnt_mat.max(axis=(0, 2)), 2)
    K_t = K_t + (K_t % 2)
    koff = np.concatenate([[0], np.cumsum(K_t)])
    ksum = int(koff[-1])

    # layer-1 payload values: fp16 x0 by token, zeros at dummies
    t16 = np.zeros((tokv, D), np.float16)
    real = node_of_tok >= 0
    t16[real] = nodes[node_of_tok[real]].astype(np.float16)

    e_slot = dst_tok_s % sh
    e_core = dst_tok_s // sh
    e_t = e_slot // 128
    e_r = np.arange(len(dst_tok_s)) - start_tok[dst_tok_s]

    gidx1 = np.zeros((CORES, 128, ksum), np.int64)  # token 0 = zero row
    gidx1[e_core, e_slot % 128, koff[e_t] + e_r] = src_tok_s
    pay1 = t16[gidx1]  # [CORES, 128, ksum, D]
    pay1 = np.ascontiguousarray(pay1.reshape(CORES, 128, ksum * D))

    # ---- L2 split K-grid over two table windows
    mustA_e = src_tok_s < WINB            # must use window A
    mustB_e = src_tok_s >= WINSZ          # must use window B
    cA = np.bincount(dst_tok_s[mustA_e], minlength=tokv).reshape(CORES, nt, 128)
    cB = np.bincount(dst_tok_s[mustB_e], minlength=tokv).reshape(CORES, nt, 128)
    maxA = cA.max(axis=(0, 2))
    maxB = cB.max(axis=(0, 2))
    need = np.maximum(K_t, maxA + maxB)
    need = need + (need % 2)
    K_A = maxA + (need - maxA - maxB + 1) // 2
    K_B = need - K_A
    koffA = np.concatenate([[0], np.cumsum(K_A)])
    koffB = np.concatenate([[0], np.cumsum(K_B)])
    ksumA, ksumB = int(koffA[-1]), int(koffB[-1])

    # per-node window assignment: nA = clamp(max(mustA, deg-K_B), <=K_A)
    cA_tok = cA.reshape(tokv)
    cB_tok = cB.reshape(tokv)
    KA_tok = K_A[(np.arange(tokv) % sh) // 128]
    KB_tok = K_B[(np.arange(tokv) % sh) // 128]
    nA_tok = np.maximum(cA_tok, cnt_tok - KB_tok)
    assert (nA_tok <= KA_tok).all() and (cnt_tok - nA_tok <= KB_tok).all()

    # order edges of each dst: mustA, then free, then mustB; first nA -> A
    sub = np.zeros(len(dst_tok_s), np.int8)
    sub[~mustA_e & ~mustB_e] = 1
    sub[mustB_e] = 2
    order2 = np.lexsort((sub, dst_tok_s))
    d2 = dst_tok_s[order2]
    s2 = src_tok_s[order2]
    r2 = np.arange(len(d2)) - start_tok[d2]
    toA = r2 < nA_tok[d2]
    # column index within the A / B grids
    colA = koffA[(d2 % sh) // 128] + r2
    rB = r2 - nA_tok[d2]
    colB = koffB[(d2 % sh) // 128] + rB

    # idx grids, int16, row = token - base; pads point at zero rows
    padA = 0                       # token 0 (dummy) in window A
    padB = tokv - 1 - WINB         # last dummy token in window B
    idxA = np.full((CORES, 128, ksumA), padA, np.int64)
    idxB = np.full((CORES, 128, ksumB), padB, np.int64)
    eA = toA
    idxA[d2[eA] // sh, (d2[eA] % sh) % 128, colA[eA]] = s2[eA]
    eB = ~toA
    idxB[d2[eB] // sh, (d2[eB] % sh) % 128, colB[eB]] = s2[eB] - WINB

    def pack_idx(grid, ncol):
        # [128 slots, ncols] -> dma_gather layout [128, ncols*8] int16:
        # per tile column k the 128 indices are at [(i%16), k*8 + i//16],
        # replicated into all 8 16-partition bands.
        out = np.empty((CORES, 128, ncol * 8), np.int16)
        for c in range(CORES):
            g = grid[c]  # [128, ncol]
            # i = slot p; idx i of column k at [i%16, 8k + i//16]
            a = g.T.reshape(ncol, 8, 16).transpose(2, 0, 1).reshape(16, ncol * 8)
            out[c] = np.tile(a.astype(np.int16), (8, 1))
        return out

    idxA16 = pack_idx(idxA, ksumA)
    idxB16 = pack_idx(idxB, ksumB)

    # per-core invdeg [128, nt] (0 for dummy slots)
    invdeg = np.zeros((CORES, 128, nt), np.float32)
    deg_tok = cnt_tok.reshape(CORES, sh)
    node_ok = (node_of_tok.reshape(CORES, sh) >= 0)
    iv = (1.0 / np.maximum(deg_tok, 1.0)) * node_ok
    for c in range(CORES):
        invdeg[c] = iv[c].reshape(nt, 128).T

    # per-core x0 feature-major [D, sh]
    x0_fm = np.zeros((CORES, D, sh), np.float32)
    for c in range(CORES):
        nm = node_of_tok[c * sh:(c + 1) * sh]
        ok = nm >= 0
        blk = np.zeros((sh, D), np.float32)
        blk[ok] = nodes[nm[ok]]
        x0_fm[c] = blk.T

    return dict(
        N=N, sh_real=sh_real, sh=sh, nt=nt, tokv=tokv,
        K_t=[int(k) for k in K_t], koff=[int(k) for k in koff], ksum=ksum,
        K_A=[int(k) for k in K_A], koffA=[int(k) for k in koffA], ksumA=ksumA,
        K_B=[int(k) for k in K_B], koffB=[int(k) for k in koffB], ksumB=ksumB,
        idxA16=idxA16, idxB16=idxB16, invdeg=invdeg, pay1=pay1, x0_fm=x0_fm,
        node_of_tok=node_of_tok,
    )


# ----------------------------------------------------------------------------
# Program builder
# ----------------------------------------------------------------------------

def build_program(cfg, debug=False):
    nt, sh, sh_real = cfg["nt"], cfg["sh"], cfg["sh_real"]
    tokv = cfg["tokv"]
    K_t, koff, ksum = cfg["K_t"], cfg["koff"], cfg["ksum"]
    K_A, koffA, ksumA = cfg["K_A"], cfg["koffA"], cfg["ksumA"]
    K_B, koffB, ksumB = cfg["K_B"], cfg["koffB"], cfg["ksumB"]
    N = cfg["N"]
    kmax = max(max(K_t), max(a + b for a, b in zip(K_A, K_B)))
    rg = [list(range(CORES))]

    chunks = []
    c0 = 0
    while c0 < sh:
        cw = min(CHUNK, sh - c0)
        chunks.append((c0, cw))
        c0 += cw
    nch = len(chunks)

    nc = bacc.Bacc("TRN2", target_bir_lowering=False, debug=False,
                   num_devices=CORES)

    # ---- I/O declarations
    pay1_d = nc.dram_tensor("pay1", [128, ksum * D], F16, kind="ExternalInput")
    x0_d = nc.dram_tensor("x0_fm", [D, sh], F32, kind="ExternalInput")
    idxA_d = nc.dram_tensor("idxA", [128, ksumA * 8], I16, kind="ExternalInput")
    idxB_d = nc.dram_tensor("idxB", [128, ksumB * 8], I16, kind="ExternalInput")
    invdeg_d = nc.dram_tensor("invdeg", [128, nt], F32, kind="ExternalInput")
    wg_d = [nc.dram_tensor(f"wg{l}", [D, D], BF16, kind="ExternalInput")
            for l in range(DEPTH)]
    w1_d = [nc.dram_tensor(f"w1_{l}", [D, H], BF16, kind="ExternalInput")
            for l in range(DEPTH)]
    fb1_d = [nc.dram_tensor(f"fb1_{l}", [D, H // D], F32, kind="ExternalInput")
             for l in range(DEPTH)]
    w2_d = [nc.dram_tensor(f"w2_{l}", [H, D], BF16, kind="ExternalInput")
            for l in range(DEPTH)]
    bn_d = {}
    for l in range(DEPTH):
        for nm in ("g1", "b1", "g2", "b2"):
            bn_d[(nm, l)] = nc.dram_tensor(f"{nm}_{l}", [D, 1], F32,
                                           kind="ExternalInput")
    clsw_d = nc.dram_tensor("clsw", [D, 16], F32, kind="ExternalInput")
    clsb_d = nc.dram_tensor("clsb", [16, 1], F32, kind="ExternalInput")
    out_d = nc.dram_tensor("out_fm", [16, sh], F32, kind="ExternalOutput")
    dbg = {}
    if debug:
        for nm, shape, dt_ in [("dbg_u0", [D, sh], F32),
                               ("dbg_v0", [D, sh], F32),
                               ("dbg_u1", [D, sh], F32),
                               ("dbg_vtab", [tokv, D], F16),
                               ("dbg_g16a", [128, kmax * D], F16),
                               ("dbg_g16b", [128, kmax * D], F16)]:
            dbg[nm] = nc.dram_tensor(nm, shape, dt_, kind="ExternalOutput")

    with tile.TileContext(nc) as tc, ExitStack() as ctx:
        dram = ctx.enter_context(tc.tile_pool(name="dram", bufs=1, space="DRAM"))
        wp = ctx.enter_context(tc.tile_pool(name="weights", bufs=1))
        big = ctx.enter_context(tc.tile_pool(name="big", bufs=1))
        gp = ctx.enter_context(tc.tile_pool(name="gather", bufs=2))
        g8p = ctx.enter_context(tc.tile_pool(name="g8", bufs=2))
        ck3 = ctx.enter_context(tc.tile_pool(name="aggc", bufs=3))
        ckx = ctx.enter_context(tc.tile_pool(name="ckx", bufs=2))
        sp = ctx.enter_context(tc.tile_pool(name="small", bufs=4))
        psR = ctx.enter_context(tc.tile_pool(name="psR", bufs=2, space="PSUM"))
        psM = ctx.enter_context(tc.tile_pool(name="psM", bufs=2, space="PSUM"))
        psP = ctx.enter_context(tc.tile_pool(name="psP", bufs=2, space="PSUM"))
        psY = ctx.enter_context(tc.tile_pool(name="psY", bufs=2, space="PSUM"))

        # ---- internal DRAM (collective bounce buffers)
        vshard = dram.tile([sh, D], F16, name="vshard")
        vtab = dram.tile([tokv, D], F16, addr_space="Shared", name="vtab")
        bn_in, bn_out = {}, {}
        for l in range(DEPTH):
            for j in (1, 2):
                bn_in[(l, j)] = dram.tile([D, 2], F32, name=f"bni{l}{j}")
                bn_out[(l, j)] = dram.tile([D, 2], F32, addr_space="Shared",
                                           name=f"bno{l}{j}")

        # ---- load constants / weights to SBUF (scalar = HWDGE ring B,
        # leaving the sync ring free for the pay1 stream)
        def load(dt_, shape, src, name):
            t = wp.tile(shape, dt_, name=name)
            nc.scalar.dma_start(out=t[:], in_=src)
            return t

        idxA_sb = load(I16, [128, ksumA * 8], idxA_d[:], "idxA_sb")
        idxB_sb = load(I16, [128, ksumB * 8], idxB_d[:], "idxB_sb")
        invdeg_sb = load(F32, [128, nt], invdeg_d[:], "invdeg_sb")
        wg_sb = [load(BF16, [D, D], wg_d[l][:], f"wg_sb{l}")
                 for l in range(DEPTH)]
        w1_sb = [load(BF16, [D, H], w1_d[l][:], f"w1_sb{l}")
                 for l in range(DEPTH)]
        fb1_sb = [load(F32, [D, H // D], fb1_d[l][:], f"fb1_sb{l}")
                  for l in range(DEPTH)]
        w2_sb = [[load(BF16, [D, D], w2_d[l][h * D:(h + 1) * D, :],
                       f"w2_sb{l}_{h}") for h in range(H // D)]
                 for l in range(DEPTH)]
        bn_sb = {k: load(F32, [D, 1], v[:], f"bn_{k[0]}_{k[1]}")
                 for k, v in bn_d.items()}
        clsw_sb = load(F32, [D, 16], clsw_d[:], "clsw_sb")
        clsb_sb = load(F32, [16, 1], clsb_d[:], "clsb_sb")
        wg1p_sb = wp.tile([D, D], BF16, name="wg1p")     # diag(a2) @ W_gcn[1]
        clswp_sb = wp.tile([D, 16], BF16, name="clswp")  # diag(a2') @ cls_w
        biasF_sb = wp.tile([16, 1], F32, name="biasF")   # c2' @ cls_w + cls_b

        ident16 = wp.tile([128, 128], F16, name="ident16")
        make_identity(nc, ident16[:])
        ident32 = wp.tile([128, 128], F32, name="ident32")
        make_identity(nc, ident32[:])

        # diag(invdeg) per tile, fp16
        diag_all = wp.tile([128, nt * 128], F16, name="diag_all")
        for t in range(nt):
            nc.vector.tensor_scalar_mul(
                out=diag_all[:, t * 128:(t + 1) * 128], in0=ident16[:],
                scalar1=invdeg_sb[:, t:t + 1])

        # ---- persistent activations (feature-major [D, sh] fp32)
        xres = big.tile([D, sh], F32, name="xres")  # x0, then xnew(l0)
        ubuf = big.tile([D, sh], F32, name="ubuf")
        vbuf = big.tile([D, sh], F32, name="vbuf")
        nc.scalar.dma_start(out=xres[:], in_=x0_d[:])

        def bn_vec_math(sums_sb, g_sb, b_sb, a_out, c_out, tag):
            """a = g*rsqrt(var+eps); c = b - mean*a, from [D,2] (sum, sumsq)."""
            m = sp.tile([D, 1], F32, tag="bnv", name=f"m{tag}")
            msq = sp.tile([D, 1], F32, tag="bnv", name=f"msq{tag}")
            var = sp.tile([D, 1], F32, tag="bnv", name=f"var{tag}")
            r = sp.tile([D, 1], F32, tag="bnv", name=f"r{tag}")
            nc.vector.tensor_scalar_mul(out=m[:], in0=sums_sb[:, 0:1],
                                        scalar1=1.0 / N)
            nc.vector.tensor_scalar_mul(out=msq[:], in0=sums_sb[:, 1:2],
                                        scalar1=1.0 / N)
            nc.vector.tensor_tensor(out=var[:], in0=m[:], in1=m[:], op=ALU.mult)
            nc.vector.tensor_tensor(out=var[:], in0=msq[:], in1=var[:],
                                    op=ALU.subtract)
            nc.vector.tensor_scalar_add(out=var[:], in0=var[:], scalar1=EPS)
            nc.vector.reciprocal(out=r[:], in_=var[:])
            nc.scalar.activation(out=a_out[:], in_=r[:], func=AF.Sqrt)
            nc.vector.tensor_tensor(out=a_out[:], in0=g_sb[:], in1=a_out[:],
                                    op=ALU.mult)
            nc.vector.tensor_tensor(out=c_out[:], in0=m[:], in1=a_out[:],
                                    op=ALU.mult)
            nc.vector.tensor_tensor(out=c_out[:], in0=b_sb[:], in1=c_out[:],
                                    op=ALU.subtract)

        def allreduce_stats(ssum, ssq, l, j, a_out, c_out):
            s2 = sp.tile([D, 2], F32, tag="s2", name=f"s2_{l}{j}")
            nc.vector.tensor_reduce(out=s2[:, 0:1], in_=ssum[:],
                                    axis=mybir.AxisListType.X, op=ALU.add)
            nc.vector.tensor_reduce(out=s2[:, 1:2], in_=ssq[:],
                                    axis=mybir.AxisListType.X, op=ALU.add)
            nc.sync.dma_start(out=bn_in[(l, j)][:], in_=s2[:])
            nc.gpsimd.collective_compute(
                "AllReduce", ALU.add, replica_groups=rg,
                ins=[bn_in[(l, j)][:]], outs=[bn_out[(l, j)][:]])
            sums = sp.tile([D, 2], F32, tag="s2", name=f"sums{l}{j}")
            nc.sync.dma_start(out=sums[:], in_=bn_out[(l, j)][:])
            bn_vec_math(sums, bn_sb[(f"g{j}", l)], bn_sb[(f"b{j}", l)],
                        a_out, c_out, f"{l}{j}")

        def emit_gather(l, t, G16):
            """Fill G16[:, :W*D] with the tile's neighbor rows (fp16)."""
            if l == 0:
                K = K_t[t]
                nc.sync.dma_start(
                    out=G16[:, :K * D],
                    in_=pay1_d[:, koff[t] * D:(koff[t] + K) * D])
                return K
            KA, KB = K_A[t], K_B[t]
            if KA:
                nc.gpsimd.dma_gather(
                    out_ap=G16[:, :KA * D].rearrange(
                        "p (g d) -> p g d", g=KA, d=D),
                    in_ap=vtab[0:WINSZ, :],
                    idxs_ap=idxA_sb[:, koffA[t] * 8:(koffA[t] + KA) * 8],
                    num_idxs=KA * 128, num_idxs_reg=KA * 128, elem_size=D,
                    single_packet=False)
            if KB:
                nc.gpsimd.dma_gather(
                    out_ap=G16[:, KA * D:(KA + KB) * D].rearrange(
                        "p (g d) -> p g d", g=KB, d=D),
                    in_ap=vtab[WINB:WINB + WINSZ, :],
                    idxs_ap=idxB_sb[:, koffB[t] * 8:(koffB[t] + KB) * 8],
                    num_idxs=KB * 128, num_idxs_reg=KB * 128, elem_size=D,
                    single_packet=False)
            return KA + KB

        for l in range(DEPTH):
            # ---------- aggregation + GCN sweep, pipelined per chunk
            ssum1 = sp.tile([D, nch], F32, tag="st1", name=f"ssum1_{l}")
            ssq1 = sp.tile([D, nch], F32, tag="st1", name=f"ssq1_{l}")
            for ci, (c0, cw) in enumerate(chunks):
                tiles = range(c0 // 128, (c0 + cw) // 128)
                aggc = ck3.tile([D, CHUNK], BF16, tag="aggc",
                                name=f"aggc{l}_{ci}")
                for t in tiles:
                    G16 = gp.tile([128, kmax * D], F16, tag="G16",
                                  name=f"G{l}_{t}")
                    W = emit_gather(l, t, G16)
                    if dbg and l == 1 and t in (5, 30):
                        nc.sync.dma_start(
                            out=dbg["dbg_g16a" if t == 5 else "dbg_g16b"][:, :W * D],
                            in_=G16[:, :W * D])
                    # fp16 pairwise passes: W -> W/2 (-> W/4 when W%4==0)
                    h2 = W // 2
                    G8 = g8p.tile([128, (kmax // 2) * D], F16, tag="G8",
                                  name=f"G8_{l}_{t}")
                    nc.vector.tensor_tensor(
                        out=G8[:, :h2 * D], in0=G16[:, :h2 * D],
                        in1=G16[:, h2 * D:W * D], op=ALU.add)
                    if h2 % 2 == 0:
                        h4 = h2 // 2
                        G4 = g8p.tile([128, (kmax // 4) * D], F16, tag="G4",
                                      name=f"G4_{l}_{t}")
                        nc.vector.tensor_tensor(
                            out=G4[:, :h4 * D], in0=G8[:, :h4 * D],
                            in1=G8[:, h4 * D:h2 * D], op=ALU.add)
                        red_src, nred = G4, h4
                    else:
                        red_src, nred = G8, h2
                    ps = psR.tile([D, 128], F32, tag="red", name=f"red{l}_{t}")
                    dg = diag_all[:, t * 128:(t + 1) * 128]
                    for k in range(nred):
                        nc.tensor.matmul(ps[:], red_src[:, k * D:(k + 1) * D],
                                         dg, start=(k == 0),
                                         stop=(k == nred - 1))
                    nc.scalar.activation(
                        out=aggc[:, (t - c0 // 128) * 128:
                                 (t - c0 // 128 + 1) * 128],
                        in_=ps[:], func=AF.Copy)
                # GCN linear + residual -> u
                sl = slice(c0, c0 + cw)
                wgl = wg_sb[0] if l == 0 else wg1p_sb
                ph = psM.tile([D, CHUNK], F32, tag="mm1", name=f"ph{l}{c0}")
                nc.tensor.matmul(ph[:, :cw], wgl[:], aggc[:, :cw],
                                 start=True, stop=True)
                nc.vector.tensor_tensor(out=ubuf[:, sl], in0=ph[:, :cw],
                                        in1=xres[:, sl], op=ALU.add)
                # BN1 partial stats over real columns (slot 0 is a dummy but
                # u(dummy) == 0 so including it is harmless)
                rw = max(0, min(cw, sh_real + 1 - c0))
                if rw == 0:
                    nc.vector.memset(ssum1[:, ci:ci + 1], 0.0)
                    nc.vector.memset(ssq1[:, ci:ci + 1], 0.0)
                else:
                    nc.vector.tensor_reduce(out=ssum1[:, ci:ci + 1],
                                            in_=ubuf[:, c0:c0 + rw],
                                            axis=mybir.AxisListType.X,
                                            op=ALU.add)
                    sqs = ckx.tile([D, CHUNK], F32, tag="sqs",
                                   name=f"sq1{l}{ci}")
                    nc.scalar.activation(out=sqs[:, :rw],
                                         in_=ubuf[:, c0:c0 + rw],
                                         func=AF.Square,
                                         accum_out=ssq1[:, ci:ci + 1])
            if dbg and l == 0:
                nc.sync.dma_start(out=dbg["dbg_u0"][:], in_=ubuf[:])
            if dbg and l == 1:
                nc.sync.dma_start(out=dbg["dbg_u1"][:], in_=ubuf[:])
            a1 = sp.tile([D, 1], F32, tag="co", name=f"a1_{l}")
            c1 = sp.tile([D, 1], F32, tag="co", name=f"c1_{l}")
            allreduce_stats(ssum1, ssq1, l, 1, a1, c1)

            # ---------- FF sweep (+ layer-0: v transposes & AllGather)
            ssum2 = sp.tile([D, nch], F32, tag="st2", name=f"ssum2_{l}")
            ssq2 = sp.tile([D, nch], F32, tag="st2", name=f"ssq2_{l}")
            next_tp = 0
            for ci, (c0, cw) in enumerate(chunks):
                sl = slice(c0, c0 + cw)
                xp = ckx.tile([D, CHUNK], F32, tag="xp", name=f"xp{l}{c0}")
                nc.vector.tensor_scalar(out=xp[:, :cw], in0=ubuf[:, sl],
                                        scalar1=a1[:], scalar2=c1[:],
                                        op0=ALU.mult, op1=ALU.add)
                xpb = ckx.tile([D, CHUNK], BF16, tag="xpb", name=f"xb{l}{c0}")
                nc.scalar.activation(out=xpb[:, :cw], in_=ubuf[:, sl],
                                     func=AF.Identity, bias=c1[:],
                                     scale=a1[:])
                py = psY.tile([D, CHUNK], F32, tag="py", name=f"py{l}{c0}")
                for h in range(H // D):
                    pr = psP.tile([128, CHUNK], F32, tag="pr",
                                  name=f"pr{l}{c0}{h}")
                    nc.tensor.matmul(pr[:, :cw],
                                     w1_sb[l][:, h * D:(h + 1) * D],
                                     xpb[:, :cw], start=True, stop=True)
                    rh = ckx.tile([128, CHUNK], BF16, tag="rh",
                                  name=f"rh{l}{c0}{h}")
                    nc.scalar.activation(out=rh[:, :cw], in_=pr[:, :cw],
                                         func=AF.Relu,
                                         bias=fb1_sb[l][:, h:h + 1], scale=1.0)
                    nc.tensor.matmul(py[:, :cw], w2_sb[l][h][:], rh[:, :cw],
                                     start=(h == 0), stop=(h == H // D - 1))
                nc.vector.tensor_tensor(out=vbuf[:, sl], in0=py[:, :cw],
                                        in1=xp[:, :cw], op=ALU.add)
                # BN2 partial stats (DVE); must exclude dummies: v(dummy) != 0
                lo = c0 + 1 if c0 == 0 else c0  # skip slot 0
                hi = min(c0 + cw, sh_real + 1)
                if hi <= lo:
                    nc.vector.memset(ssum2[:, ci:ci + 1], 0.0)
                    nc.vector.memset(ssq2[:, ci:ci + 1], 0.0)
                else:
                    nc.vector.tensor_reduce(out=ssum2[:, ci:ci + 1],
                                            in_=vbuf[:, lo:hi],
                                            axis=mybir.AxisListType.X,
                                            op=ALU.add)
                    sqv = ckx.tile([D, CHUNK], F32, tag="sqs",
                                   name=f"sq2{l}{ci}")
                    nc.vector.tensor_tensor(out=sqv[:, :hi - lo],
                                            in0=vbuf[:, lo:hi],
                                            in1=vbuf[:, lo:hi],
                                            op=ALU.mult)
                    nc.vector.tensor_reduce(out=ssq2[:, ci:ci + 1],
                                            in_=sqv[:, :hi - lo],
                                            axis=mybir.AxisListType.X,
                                            op=ALU.add)
                if l == 0:
                    # zero dummy columns before tiles are shipped
                    if ci == 0:
                        nc.vector.memset(vbuf[:, 0:1], 0.0)
                    if ci == nch - 1 and sh > sh_real + 1:
                        nc.vector.memset(vbuf[:, sh_real + 1:sh], 0.0)
                    done_t = (c0 + cw) // 128
                    while next_tp < done_t:
                        t = next_tp
                        pt = psR.tile([128, 128], F32, tag="red",
                                      name=f"vt{t}")
                        nc.tensor.transpose(pt[:],
                                            vbuf[:, t * 128:(t + 1) * 128],
                                            ident32[:])
                        vT = sp.tile([128, D], F16, tag="vT", name=f"vT{t}")
                        nc.scalar.activation(out=vT[:], in_=pt[:], func=AF.Copy)
                        nc.scalar.dma_start(
                            out=vshard[t * 128:(t + 1) * 128, :], in_=vT[:])
                        next_tp += 1
                    if next_tp == nt:
                        nc.gpsimd.collective_compute(
                            "AllGather", ALU.bypass, replica_groups=rg,
                            ins=[vshard[:]], outs=[vtab[:]])
                        next_tp += 1
            if dbg and l == 0:
                nc.sync.dma_start(out=dbg["dbg_v0"][:], in_=vbuf[:])
                nc.sync.dma_start(out=dbg["dbg_vtab"][:], in_=vtab[:])
            a2 = sp.tile([D, 1], F32, tag="co", name=f"a2_{l}")
            c2 = sp.tile([D, 1], F32, tag="co", name=f"c2_{l}")
            allreduce_stats(ssum2, ssq2, l, 2, a2, c2)

            if l == 0:
                # fold BN2 affine into layer-2 GCN weight; local xnew residual
                nc.vector.tensor_scalar_mul(out=wg1p_sb[:], in0=wg_sb[1][:],
                                            scalar1=a2[:])
                for c0, cw in chunks:
                    sl = slice(c0, c0 + cw)
                    nc.vector.tensor_scalar(out=xres[:, sl], in0=vbuf[:, sl],
                                            scalar1=a2[:], scalar2=c2[:],
                                            op0=ALU.mult, op1=ALU.add)
            else:
                # classifier with BN2 folded in:
                # logits = v2 @ (diag(a2) clsw) + (c2 @ clsw + clsb)
                nc.vector.tensor_scalar_mul(out=clswp_sb[:], in0=clsw_sb[:],
                                            scalar1=a2[:])
                pb = psM.tile([16, CHUNK], F32, tag="mm1", name="pb")
                nc.tensor.matmul(pb[:, 0:1], clsw_sb[:], c2[:],
                                 start=True, stop=True)
                nc.vector.tensor_tensor(out=biasF_sb[:], in0=pb[:, 0:1],
                                        in1=clsb_sb[:], op=ALU.add)
                out_sb = wp.tile([16, sh], F32, name="out_sb")
                for c0, cw in chunks:
                    sl = slice(c0, c0 + cw)
                    vb = ckx.tile([D, CHUNK], BF16, tag="vb", name=f"vb{c0}")
                    nc.scalar.activation(out=vb[:, :cw], in_=vbuf[:, sl],
                                         func=AF.Copy)
                    pc = psM.tile([16, CHUNK], F32, tag="mm1", name=f"pc{c0}")
                    nc.tensor.matmul(pc[:, :cw], clswp_sb[:], vb[:, :cw],
                                     start=True, stop=True)
                    nc.scalar.activation(out=out_sb[:, sl], in_=pc[:, :cw],
                                         func=AF.Identity, bias=biasF_sb[:],
                                         scale=1.0)
                    nc.sync.dma_start(out=out_d[:, sl], in_=out_sb[:, sl])

    nc.compile()
    return nc


# ----------------------------------------------------------------------------
# Entry points
# ----------------------------------------------------------------------------

def _make_in_maps(cfg, inputs):
    W_gcn = np.asarray(inputs["W_gcn"], np.float32)
    ff_w1 = np.asarray(inputs["ff_w1"], np.float32)
    ff_b1 = np.asarray(inputs["ff_b1"], np.float32)
    ff_w2 = np.asarray(inputs["ff_w2"], np.float32)
    cls_w = np.asarray(inputs["cls_w"], np.float32)
    cls_b = np.asarray(inputs["cls_b"], np.float32)

    shared = {
        "clsw": np.ascontiguousarray(cls_w),
        "clsb": np.ascontiguousarray(cls_b.reshape(16, 1)),
    }
    for l in range(DEPTH):
        shared[f"wg{l}"] = _bf16(W_gcn[l])
        shared[f"w1_{l}"] = _bf16(ff_w1[l])
        shared[f"fb1_{l}"] = np.ascontiguousarray(
            ff_b1[l].reshape(H // D, D).T)
        shared[f"w2_{l}"] = _bf16(ff_w2[l])
        shared[f"g1_{l}"] = np.ascontiguousarray(
            np.asarray(inputs["bn1_g"], np.float32)[l].reshape(D, 1))
        shared[f"b1_{l}"] = np.ascontiguousarray(
            np.asarray(inputs["bn1_b"], np.float32)[l].reshape(D, 1))
        shared[f"g2_{l}"] = np.ascontiguousarray(
            np.asarray(inputs["bn2_g"], np.float32)[l].reshape(D, 1))
        shared[f"b2_{l}"] = np.ascontiguousarray(
            np.asarray(inputs["bn2_b"], np.float32)[l].reshape(D, 1))

    in_maps = []
    for c in range(CORES):
        m = dict(shared)
        m["x0_fm"] = np.ascontiguousarray(cfg["x0_fm"][c])
        m["pay1"] = cfg["pay1"][c]
        m["idxA"] = np.ascontiguousarray(cfg["idxA16"][c])
        m["idxB"] = np.ascontiguousarray(cfg["idxB16"][c])
        m["invdeg"] = np.ascontiguousarray(cfg["invdeg"][c])
        in_maps.append(m)
    return in_maps


def _postprocess(cfg, results):
    sh = cfg["sh"]
    N = cfg["N"]
    node_of_tok = cfg["node_of_tok"]
    out = np.empty((N, 16), np.float32)
    for c in range(CORES):
        arr = results[c]["out_fm"]  # [16, sh]
        toks = np.arange(c * sh, (c + 1) * sh)
        sel = node_of_tok[toks] >= 0
        out[node_of_tok[toks[sel]]] = arr.T[sel]
    return out


def _ensure_axon_hooks():
    """The agent image's antenv lacks axon_hooks; synthesize it so
    bass_utils' trace=True path can find the NTFF profile hook."""
    try:
        import antenv.axon_hooks  # noqa: F401
        return
    except ImportError:
        pass
    import types
    import antenv
    mod = types.ModuleType("antenv.axon_hooks")
    mod._hook = None

    def set_axon_ntff_profile_hook(h):
        mod._hook = h

    def get_axon_ntff_profile_hook():
        return mod._hook

    mod.set_axon_ntff_profile_hook = set_axon_ntff_profile_hook
    mod.get_axon_ntff_profile_hook = get_axon_ntff_profile_hook
    sys.modules["antenv.axon_hooks"] = mod
    antenv.axon_hooks = mod
    try:
        from trn_agent_boot.trn_boot import _ntff_profile_via_ctypes
        h = _ntff_profile_via_ctypes("/opt/axon/libaxon_pjrt.so")
        if h is not None:
            mod._hook = h
    except Exception as e:  # pragma: no cover
        print(f"ntff hook setup failed: {e}", file=sys.stderr)


_CACHE = {}


def run(trace=False, debug=False, **inputs):
    if trace:
        _ensure_axon_hooks()
    nodes = np.asarray(inputs["nodes"], np.float32)
    edge_src = np.asarray(inputs["edge_src"], np.int64)
    edge_dst = np.asarray(inputs["edge_dst"], np.int64)
    cfg = _prepare(nodes, edge_src, edge_dst)

    key = (nodes.shape, len(edge_src), tuple(cfg["K_t"]), debug)
    if key not in _CACHE:
        _CACHE[key] = build_program(cfg, debug=debug)
    nc = _CACHE[key]

    in_maps = _make_in_maps(cfg, inputs)
    res = run_bass_kernel_spmd(nc, in_maps, list(range(CORES)), trace=trace)
    return _postprocess(cfg, res.results), res


def kernel(**inputs) -> np.ndarray:
    out, _ = run(trace=False, **inputs)
    return out
